# revision 1
# baseline (speedup 1.0000x reference)
"""Causal self-attention with RoPE on 8 TRN2 NeuronCores — v2.

Sharding: core c -> (batch b = c//4, head-group g = c%4; 4 heads of 128 each).
Tensor-parallel over heads x data-parallel over batch.

v2 strategy vs baseline:
  - bf16 compute throughout (inputs pre-converted on host).
  - single fused pass per 512-token chunk: x loaded once, K/Q/V computed
    together; K/V stay in SBUF (no DRAM spill), Q transient per chunk.
  - projection restructured: each core computes a FULL-WIDTH partial
    z_part = Wp[rows g].T @ y_local; a ReduceScatter(add) sums partials and
    scatters e-column slices -- replaces the 4x-more-expensive AllGather.
  - per-head RoPE chains so attention never waits on a rope DMA.
  - attention emitted with 2-tile lookahead so PE never waits on the exp.
  - batched 3-D-AP DMA loads; startup ordered so the first K matmuls can
    begin after just wk + the first slice of x.
  - last chunk's projection + ReduceScatter split in token halves to
    shorten the end-of-kernel collective tail.
"""
from contextlib import ExitStack

import numpy as np
import ml_dtypes

import concourse.bass as bass
import concourse.tile as tile
import concourse.mybir as mybir
from concourse import bacc, bass_utils

B = 2
S = 2048
D = 2048
NH, HD = 16, 128
HPC = 4                 # heads per core
EL = HPC * HD           # 512: local e-width per core
CH = 512                # token-chunk width
NCH = S // CH           # 4
DT = D // 128           # 16 d-tiles
ROPE_THETA = 10000.0
N_CORES = 8

F32 = mybir.dt.float32
F32R = mybir.dt.float32r
BF16 = mybir.dt.bfloat16
AF = mybir.ActivationFunctionType


def _build():
    nc = bacc.Bacc("TRN2", target_bir_lowering=False, debug=False,
                   enable_asserts=True, num_devices=N_CORES)
    xT = nc.dram_tensor("xT", [D, S], BF16, kind="ExternalInput").ap()
    wq = nc.dram_tensor("wq", [D, EL], BF16, kind="ExternalInput").ap()
    wk = nc.dram_tensor("wk", [D, EL], BF16, kind="ExternalInput").ap()
    wv = nc.dram_tensor("wv", [D, EL], BF16, kind="ExternalInput").ap()
    wp = nc.dram_tensor("wp", [EL, D], BF16, kind="ExternalInput").ap()
    cosT = nc.dram_tensor("cosT", [HD, S], BF16, kind="ExternalInput").ap()
    sinT = nc.dram_tensor("sinT", [HD, S], BF16, kind="ExternalInput").ap()
    tri = nc.dram_tensor("tri", [128, 128], BF16, kind="ExternalInput").ap()
    ones = nc.dram_tensor("ones", [128, 1], BF16, kind="ExternalInput").ap()
    onesT = nc.dram_tensor("onesT", [1, 128], F32R, kind="ExternalInput").ap()
    zTc = nc.dram_tensor("zTc", [NCH * EL, CH], BF16, kind="ExternalOutput").ap()

    with tile.TileContext(nc) as tc, \
         nc.allow_low_precision(reason="bf16 attention"), ExitStack() as ctx:
        # ---------------- pools ----------------
        cpool = ctx.enter_context(tc.tile_pool(name="const", bufs=1))
        wpool = ctx.enter_context(tc.tile_pool(name="w", bufs=1))
        xpool = ctx.enter_context(tc.tile_pool(name="x", bufs=2))
        kvres = ctx.enter_context(tc.tile_pool(name="kv", bufs=1))
        qpool = ctx.enter_context(tc.tile_pool(name="q", bufs=2))
        rope = ctx.enter_context(tc.tile_pool(name="rope", bufs=8))
        ppool = ctx.enter_context(tc.tile_pool(name="p", bufs=6))
        ypool = ctx.enter_context(tc.tile_pool(name="y", bufs=2))
        rpool = ctx.enter_context(tc.tile_pool(name="r", bufs=2))
        bpool = ctx.enter_context(tc.tile_pool(name="rbc", bufs=1))
        dram = ctx.enter_context(tc.tile_pool(name="dram", bufs=1, space="DRAM"))
        ps_mm = ctx.enter_context(tc.tile_pool(name="ps_mm", bufs=2, space="PSUM"))
        ps_s = ctx.enter_context(tc.tile_pool(name="ps_s", bufs=3, space="PSUM"))
        ps_o = ctx.enter_context(tc.tile_pool(name="ps_o", bufs=2, space="PSUM"))
        ps_r = ctx.enter_context(tc.tile_pool(name="ps_r", bufs=1, space="PSUM"))

        # ------------- weight / x loaders (split DMAs for pipelining) -------
        WSPLIT = 4            # d-tiles per weight sub-DMA

        def load_w(name, src, nt, wcols, nsub):
            t = wpool.tile([128, nt * wcols], BF16, name=name)
            step = nt // nsub
            for i in range(nsub):
                nc.sync.dma_start(
                    t[:, i * step * wcols:(i + 1) * step * wcols]
                        .rearrange("p (t e) -> p t e", t=step),
                    src.rearrange("(t p) e -> p t e", p=128)[:, i * step:(i + 1) * step, :])
            return t

        def load_x(ci):
            xt = xpool.tile([128, DT * CH], BF16, tag="x", name=f"x{ci}")
            nsub, step = 4, DT // 4
            for i in range(nsub):
                nc.sync.dma_start(
                    xt[:, i * step * CH:(i + 1) * step * CH]
                        .rearrange("p (t c) -> p t c", t=step),
                    xT.rearrange("(t p) s -> p t s", p=128)
                      [:, i * step:(i + 1) * step, ci * CH:(ci + 1) * CH])
            return xt

        # startup order: wk/x(0) interleaved (K matmuls start first), then
        # cos/sin (K rope), wq, wv, then attention constants.
        wk_sb = wpool.tile([128, DT * EL], BF16, name="wk_sb")
        x_cur = xpool.tile([128, DT * CH], BF16, tag="x", name="x0")
        for (i0, i1) in [(0, 1), (1, 4), (4, 8), (8, 12), (12, 16)]:
            nc.sync.dma_start(
                wk_sb[:, i0 * EL:i1 * EL].rearrange("p (t e) -> p t e", t=i1 - i0),
                wk.rearrange("(t p) e -> p t e", p=128)[:, i0:i1, :])
            nc.sync.dma_start(
                x_cur[:, i0 * CH:i1 * CH].rearrange("p (t c) -> p t c", t=i1 - i0),
                xT.rearrange("(t p) s -> p t s", p=128)[:, i0:i1, 0:CH])
        cos_t = cpool.tile([HD, S], BF16)
        nc.sync.dma_start(cos_t[:], cosT)
        sin_t = cpool.tile([HD, S], BF16)
        nc.sync.dma_start(sin_t[:], sinT)
        wq_sb = load_w("wq", wq, DT, EL, WSPLIT)
        wv_sb = load_w("wv", wv, DT, EL, WSPLIT)
        tri_t = cpool.tile([128, 128], BF16)
        nc.sync.dma_start(tri_t[:], tri)
        ones_t = cpool.tile([128, 1], BF16)
        nc.sync.dma_start(ones_t[:], ones)
        onesT_t = cpool.tile([1, 128], F32R)
        nc.sync.dma_start(onesT_t[:], onesT)

        # ---------------- persistent K / V, z scratch ----------------
        k_c = [kvres.tile([HD, HPC * CH], BF16, name=f"k{ci}") for ci in range(NCH)]
        v_t = [kvres.tile([128, EL], BF16, name=f"v{st}") for st in range(S // 128)]
        z_shapes = [[(0, CH)]] * NCH
        z_part = {}
        z_rs = {}
        for ci in range(NCH):
            for (c0, c1) in z_shapes[ci]:
                z_part[(ci, c0)] = dram.tile([D, c1 - c0], BF16,
                                             tag=f"zp{ci}_{c0}", name=f"zp{ci}_{c0}")
                z_rs[(ci, c0)] = dram.tile([EL, c1 - c0], BF16,
                                           tag=f"zr{ci}_{c0}", name=f"zr{ci}_{c0}")

        def kq_head_mms(w_sb, x_sb, h, ps):
            for dt in range(DT):
                nc.tensor.matmul(
                    ps[:], w_sb[:, dt * EL + h * HD:dt * EL + (h + 1) * HD],
                    x_sb[:, dt * CH:(dt + 1) * CH],
                    start=(dt == 0), stop=(dt == DT - 1))

        _QKV_POOLS = [(ps_s, "s_ps"), (ps_o, "o"), (ps_mm, "ps")]
        _qkv_rot = [0]

        def qkv_psum(name):
            pool, tag = _QKV_POOLS[_qkv_rot[0] % 3]
            _qkv_rot[0] += 1
            return pool.tile([128, CH], F32, tag=tag, name=name)

        def rope_head(ci, h, x_sb, w_sb, out_ap, tagp):
            """One head's [HD, CH] projection + RoPE -> out_ap."""
            ps = qkv_psum(f"ps_{tagp}")
            kq_head_mms(w_sb, x_sb, h, ps)
            pre = rope.tile([HD, CH], BF16, tag="pre", name=f"pre_{tagp}")
            nc.scalar.copy(pre[:], ps[:])
            rot = rope.tile([HD, CH], BF16, tag="rot", name=f"rot_{tagp}")
            nc.sync.dma_start(rot[0:64, :], pre[64:128, :])
            nc.sync.dma_start(rot[64:128, :], pre[0:64, :])
            cs = cos_t[:, ci * CH:(ci + 1) * CH]
            sn = sin_t[:, ci * CH:(ci + 1) * CH]
            t1 = rope.tile([HD, CH], BF16, tag="t1", name=f"t1_{tagp}")
            t2 = rope.tile([HD, CH], BF16, tag="t2", name=f"t2_{tagp}")
            nc.vector.tensor_mul(t1[:], pre[:], cs)
            nc.vector.tensor_mul(t2[:], rot[:], sn)
            nc.vector.tensor_add(out_ap, t1[:], t2[:])

        def qkv_chunk(ci, x_sb):
            q_sb = qpool.tile([128, HPC * CH], BF16, tag="q", name=f"q{ci}")
            # K, Q, then V: the trailing ACT evacs at attention start are V's,
            # which attention only needs at the (late) diagonal tiles — the
            # first exps never queue behind an evac.
            for h in range(HPC):
                rope_head(ci, h, x_sb, wk_sb,
                          k_c[ci][:, h * CH:(h + 1) * CH], f"k{ci}_{h}")
            for h in range(HPC):
                rope_head(ci, h, x_sb, wq_sb,
                          q_sb[:, h * CH:(h + 1) * CH], f"q{ci}_{h}")
            for st in range(CH // 128):
                ps = qkv_psum(f"ps_v{ci}_{st}")
                for dt in range(DT):
                    nc.tensor.matmul(
                        ps[:], x_sb[:, dt * CH + st * 128:dt * CH + (st + 1) * 128],
                        wv_sb[:, dt * EL:(dt + 1) * EL],
                        start=(dt == 0), stop=(dt == DT - 1))
                nc.scalar.copy(v_t[ci * 4 + st][:], ps[:])
            return q_sb

        def attn_chunk(ci, q_sb):
            """Causal attention for query chunk ci over key chunks 0..ci.
            2-tile lookahead emission keeps PE ahead of the exp latency."""
            y_sb = ypool.tile([128, HPC * CH], BF16, tag="y", name=f"y{ci}")
            n_jt = 4 * ci + 4
            tiles = [(h, jt) for h in range(HPC) for jt in range(n_jt)]
            state = {}
            pending = []

            def emit_or(ent):
                h, jt, p, off = ent
                o_ps, r_ps = state[h]
                nc.tensor.matmul(
                    o_ps[:, off:], v_t[jt][:, h * HD:(h + 1) * HD],
                    p[:, off:], start=(jt == 0), stop=(jt == n_jt - 1))
                nc.tensor.matmul(
                    r_ps[:, off:], ones_t[:], p[:, off:],
                    start=(jt == 0), stop=(jt == n_jt - 1))
                if jt == n_jt - 1:
                    # normalize head h: y = o * (1/rowsum); the broadcast of
                    # rinv across partitions runs on the otherwise-idle
                    # gpsimd engine instead of a PE matmul.
                    rinv = rpool.tile([1, CH], F32R, tag="rinv")
                    nc.vector.reciprocal(rinv[:], r_ps[:])
                    o_sb = rpool.tile([HD, CH], F32R, tag="osb", name=f"os{ci}_{h}")
                    nc.scalar.copy(o_sb[:], o_ps[:])
                    rbc = bpool.tile([128, CH], F32R, tag="rbc", name=f"rb{ci}_{h}")
                    nc.gpsimd.partition_broadcast(rbc[:], rinv[:])
                    nc.vector.tensor_mul(
                        y_sb[:, h * CH:(h + 1) * CH], o_sb[:], rbc[:])
                    del state[h]

            for idx, (h, jt) in enumerate(tiles):
                if jt == 0:
                    o_ps = ps_o.tile([HD, CH], F32, tag="o", name=f"o{ci}_{h}")
                    r_ps = ps_r.tile([1, CH], F32, tag="r", name=f"r{ci}_{h}")
                    state[h] = (o_ps, r_ps)
                diag = jt - 4 * ci
                off = 128 * diag if diag > 0 else 0
                cj, j2 = divmod(jt, 4)
                # score tiles alternate between ps_s and the (attention-idle)
                # ps_mm pool, giving a 5-bank rotation for deeper lookahead
                spool = ps_s if idx % 2 == 0 else ps_mm
                stag = "s_ps" if idx % 2 == 0 else "ps"
                s_ps = spool.tile([128, CH], F32, tag=stag, name=f"s{ci}_{h}_{jt}")
                nc.tensor.matmul(
                    s_ps[:, off:], k_c[cj][:, h * CH + j2 * 128:h * CH + (j2 + 1) * 128],
                    q_sb[:, h * CH + off:(h + 1) * CH], start=True, stop=True)
                p = ppool.tile([128, CH], BF16, tag="p")
                nc.scalar.activation(p[:, off:], s_ps[:, off:], AF.Exp)
                if 0 <= diag:
                    nc.vector.tensor_mul(
                        p[:, off:off + 128], p[:, off:off + 128], tri_t[:])
                if len(pending) >= 5:
                    emit_or(pending.pop(0))
                pending.append((h, jt, p, off))
            for ent in pending:
                emit_or(ent)
            return y_sb

        def proj_chunk(ci, y_sb, wp_sb):
            for (c0, c1) in z_shapes[ci]:
                cw = c1 - c0
                zp = z_part[(ci, c0)]
                for eb in range(DT):
                    pool = ps_mm if eb % 2 == 0 else ps_o
                    tag = "ps" if eb % 2 == 0 else "o"
                    ps = pool.tile([128, CH], F32, tag=tag, name=f"ps_z{ci}_{eb}")
                    for ct in range(EL // 128):
                        nc.tensor.matmul(
                            ps[:, 0:cw],
                            wp_sb[:, ct * D + eb * 128:ct * D + (eb + 1) * 128],
                            y_sb[:, ct * CH + c0:ct * CH + c1],
                            start=(ct == 0), stop=(ct == EL // 128 - 1))
                    zev = ppool.tile([128, CH], BF16, tag="zev", name=f"z{ci}_{eb}")
                    if eb % 2 == 0:
                        nc.scalar.copy(zev[:, 0:cw], ps[:, 0:cw])
                    else:
                        nc.vector.tensor_copy(zev[:, 0:cw], ps[:, 0:cw])
                    nc.sync.dma_start(zp[eb * 128:(eb + 1) * 128, :], zev[:, 0:cw])
                zr = z_rs[(ci, c0)]
                nc.gpsimd.collective_compute(
                    "ReduceScatter", mybir.AluOpType.add,
                    replica_groups=[[0, 1, 2, 3], [4, 5, 6, 7]],
                    ins=[zp.opt()], outs=[zr.opt()])

        # ---------------- main loop ----------------
        wp_sb = None
        for ci in range(NCH):
            q_sb = qkv_chunk(ci, x_cur)
            if ci + 1 < NCH:
                x_cur = load_x(ci + 1)
            if ci == 0:
                wp_sb = load_w("wp", wp, EL // 128, D, 2)
            y_sb = attn_chunk(ci, q_sb)
            proj_chunk(ci, y_sb, wp_sb)
        # RS cannot target an ExternalOutput; DRAM->DRAM DMAs move the
        # scattered slices into the output tensor. Emitted at the very end so
        # their RS-completion waits never head-of-line-block the SP DMA queue:
        # bounces 0..2 fire immediately, only the last waits on RS(3).
        for ci in range(NCH):
            for (c0, c1) in z_shapes[ci]:
                nc.sync.dma_start(zTc[ci * EL:(ci + 1) * EL, c0:c1],
                                  z_rs[(ci, c0)][:])
    nc.compile()
    return nc


def _tables():
    inv_freq = 1.0 / (ROPE_THETA ** (np.arange(0, HD, 2, dtype=np.float64) / HD))
    pos = np.arange(S, dtype=np.float64)
    f_half = np.outer(inv_freq, pos)                  # [64, S]
    freqs = np.concatenate([f_half, f_half], axis=0)  # [HD, S]
    emb32 = freqs.astype(np.float32)
    cos_t = np.cos(emb32)
    sin_t = np.sin(emb32)
    sgn = np.where(np.arange(HD) < HD // 2, -1.0, 1.0).astype(np.float32)[:, None]
    return cos_t.astype(ml_dtypes.bfloat16), (sin_t * sgn).astype(ml_dtypes.bfloat16)


_NC_CACHE = {}


def _get_nc():
    if "nc" not in _NC_CACHE:
        _NC_CACHE["nc"] = _build()
    return _NC_CACHE["nc"]


def make_in_maps(x, W_attn, W_proj):
    x = np.asarray(x, dtype=np.float32)
    W_attn = np.asarray(W_attn, dtype=np.float32)
    W_proj = np.asarray(W_proj, dtype=np.float32)
    cos_t, sin_t = _tables()
    tri = np.triu(np.ones((128, 128), np.float32)).astype(ml_dtypes.bfloat16)
    ones = np.ones((128, 1), ml_dtypes.bfloat16)
    onesT = np.ones((1, 128), np.float32)
    scale = np.float32(HD ** -0.5)
    xTb = [np.ascontiguousarray(x[b].T).astype(ml_dtypes.bfloat16) for b in range(B)]
    in_maps = []
    for c in range(N_CORES):
        b, g = divmod(c, HPC)
        in_maps.append({
            "xT": xTb[b],
            "wq": np.ascontiguousarray(
                W_attn[:, g * EL:(g + 1) * EL] * scale).astype(ml_dtypes.bfloat16),
            "wk": np.ascontiguousarray(
                W_attn[:, D + g * EL:D + (g + 1) * EL]).astype(ml_dtypes.bfloat16),
            "wv": np.ascontiguousarray(
                W_attn[:, 2 * D + g * EL:2 * D + (g + 1) * EL]).astype(ml_dtypes.bfloat16),
            "wp": np.ascontiguousarray(
                W_proj[g * EL:(g + 1) * EL, :]).astype(ml_dtypes.bfloat16),
            "cosT": cos_t, "sinT": sin_t,
            "tri": tri, "ones": ones, "onesT": onesT,
        })
    return in_maps


def assemble(results):
    out = np.empty((B, S, D), dtype=np.float32)
    for c in range(N_CORES):
        b, g = divmod(c, HPC)
        z = np.asarray(results[c]["zTc"]).astype(np.float32)   # [NCH*EL, CH]
        for ci in range(NCH):
            out[b, ci * CH:(ci + 1) * CH, g * EL:(g + 1) * EL] = \
                z[ci * EL:(ci + 1) * EL, :].T
    return out


def kernel(x, W_attn, W_proj):
    nc = _get_nc()
    in_maps = make_in_maps(x, W_attn, W_proj)
    res = bass_utils.run_bass_kernel_spmd(
        nc, in_maps, core_ids=list(range(N_CORES)), trace=False)
    return assemble(res.results)


if __name__ == "__main__":
    rng = np.random.default_rng(0)
    x = rng.standard_normal((B, S, D)).astype(np.float32)
    W_attn = (rng.standard_normal((D, 3 * D)) * D ** -0.5).astype(np.float32)
    W_proj = (rng.standard_normal((D, D)) * D ** -0.5).astype(np.float32)
    out = kernel(x, W_attn, W_proj)
    print("out", out.shape, out.dtype, np.abs(out).mean())



# revision 10
# speedup vs baseline: 1.0823x; 1.0823x over previous
"""Causal self-attention with RoPE on 8 TRN2 NeuronCores — v2.

Sharding: core c -> (batch b = c//4, head-group g = c%4; 4 heads of 128 each).
Tensor-parallel over heads x data-parallel over batch.

v2 strategy vs baseline:
  - bf16 compute throughout (inputs pre-converted on host).
  - single fused pass per 512-token chunk: x loaded once, K/Q/V computed
    together; K/V stay in SBUF (no DRAM spill), Q transient per chunk.
  - projection restructured: each core computes a FULL-WIDTH partial
    z_part = Wp[rows g].T @ y_local; a ReduceScatter(add) sums partials and
    scatters e-column slices -- replaces the 4x-more-expensive AllGather.
  - per-head RoPE chains so attention never waits on a rope DMA.
  - attention emitted with 2-tile lookahead so PE never waits on the exp.
  - batched 3-D-AP DMA loads; startup ordered so the first K matmuls can
    begin after just wk + the first slice of x.
  - last chunk's projection + ReduceScatter split in token halves to
    shorten the end-of-kernel collective tail.
"""
from contextlib import ExitStack

import numpy as np
import ml_dtypes

import concourse.bass as bass
import concourse.tile as tile
import concourse.mybir as mybir
from concourse import bacc, bass_utils

B = 2
S = 2048
D = 2048
NH, HD = 16, 128
HPC = 4                 # heads per core
EL = HPC * HD           # 512: local e-width per core
CH = 512                # token-chunk width
NCH = S // CH           # 4
DT = D // 128           # 16 d-tiles
ROPE_THETA = 10000.0
N_CORES = 8

F32 = mybir.dt.float32
F32R = mybir.dt.float32r
BF16 = mybir.dt.bfloat16
FP8 = mybir.dt.float8e4
AF = mybir.ActivationFunctionType
DR = mybir.MatmulPerfMode.DoubleRow

SX = 4.0      # fp8 scale for x
SW = 32.0     # fp8 scale for W_attn slices
QKV_SCALE = SX * SW          # q/k/v psums come out at 128x true scale
EXP_SCALE = float(HD ** -0.5)


def _build():
    nc = bacc.Bacc("TRN2", target_bir_lowering=False, debug=False,
                   enable_asserts=True, num_devices=N_CORES)
    xh = nc.dram_tensor("xh", [D, S], FP8, kind="ExternalInput").ap()
    xl = nc.dram_tensor("xl", [D, S], FP8, kind="ExternalInput").ap()
    wqh = nc.dram_tensor("wqh", [D, EL], FP8, kind="ExternalInput").ap()
    wql = nc.dram_tensor("wql", [D, EL], FP8, kind="ExternalInput").ap()
    wkh = nc.dram_tensor("wkh", [D, EL], FP8, kind="ExternalInput").ap()
    wkl = nc.dram_tensor("wkl", [D, EL], FP8, kind="ExternalInput").ap()
    wvh = nc.dram_tensor("wvh", [D, EL], FP8, kind="ExternalInput").ap()
    wvl = nc.dram_tensor("wvl", [D, EL], FP8, kind="ExternalInput").ap()
    wp = nc.dram_tensor("wp", [EL, D], BF16, kind="ExternalInput").ap()
    cosT = nc.dram_tensor("cosT", [HD, S], BF16, kind="ExternalInput").ap()
    sinT = nc.dram_tensor("sinT", [HD, S], BF16, kind="ExternalInput").ap()
    tri = nc.dram_tensor("tri", [128, 128], BF16, kind="ExternalInput").ap()
    ones = nc.dram_tensor("ones", [128, 1], BF16, kind="ExternalInput").ap()
    onesT = nc.dram_tensor("onesT", [1, 128], F32R, kind="ExternalInput").ap()
    zTc = nc.dram_tensor("zTc", [NCH * EL, CH], BF16, kind="ExternalOutput").ap()

    with tile.TileContext(nc) as tc, \
         nc.allow_low_precision(reason="bf16 attention"), ExitStack() as ctx:
        # ---------------- pools ----------------
        cpool = ctx.enter_context(tc.tile_pool(name="const", bufs=1))
        wpool = ctx.enter_context(tc.tile_pool(name="w", bufs=1))
        xpool = ctx.enter_context(tc.tile_pool(name="x", bufs=2))
        kvres = ctx.enter_context(tc.tile_pool(name="kv", bufs=1))
        qpool = ctx.enter_context(tc.tile_pool(name="q", bufs=2))
        rope = ctx.enter_context(tc.tile_pool(name="rope", bufs=8))
        ppool = ctx.enter_context(tc.tile_pool(name="p", bufs=6))
        ypool = ctx.enter_context(tc.tile_pool(name="y", bufs=2))
        rpool = ctx.enter_context(tc.tile_pool(name="r", bufs=2))
        bpool = ctx.enter_context(tc.tile_pool(name="rbc", bufs=1))
        dram = ctx.enter_context(tc.tile_pool(name="dram", bufs=1, space="DRAM"))
        ps_mm = ctx.enter_context(tc.tile_pool(name="ps_mm", bufs=2, space="PSUM"))
        ps_s = ctx.enter_context(tc.tile_pool(name="ps_s", bufs=3, space="PSUM"))
        ps_o = ctx.enter_context(tc.tile_pool(name="ps_o", bufs=2, space="PSUM"))
        ps_r = ctx.enter_context(tc.tile_pool(name="ps_r", bufs=1, space="PSUM"))

        # ------------- weight / x loaders (split DMAs for pipelining) -------
        WSPLIT = 4            # d-tiles per weight sub-DMA

        def load_w(name, src, nt, wcols, nsub):
            t = wpool.tile([128, nt * wcols], FP8, name=name)
            step = nt // nsub
            for i in range(nsub):
                nc.sync.dma_start(
                    t[:, i * step * wcols:(i + 1) * step * wcols]
                        .rearrange("p (t e) -> p t e", t=step),
                    src.rearrange("(t p) e -> p t e", p=128)[:, i * step:(i + 1) * step, :])
            return t

        def load_wp(name, src, nt, wcols, nsub):
            t = wpool.tile([128, nt * wcols], BF16, name=name)
            step = nt // nsub
            for i in range(nsub):
                nc.sync.dma_start(
                    t[:, i * step * wcols:(i + 1) * step * wcols]
                        .rearrange("p (t e) -> p t e", t=step),
                    src.rearrange("(t p) e -> p t e", p=128)[:, i * step:(i + 1) * step, :])
            return t

        def load_x(ci):
            xht = xpool.tile([128, DT * CH], FP8, tag="xh", name=f"xh{ci}")
            xlt = xpool.tile([128, DT * CH], FP8, tag="xl", name=f"xl{ci}")
            nsub, step = 2, DT // 2
            for t, src in ((xht, xh), (xlt, xl)):
                for i in range(nsub):
                    nc.sync.dma_start(
                        t[:, i * step * CH:(i + 1) * step * CH]
                            .rearrange("p (t c) -> p t c", t=step),
                        src.rearrange("(t p) s -> p t s", p=128)
                          [:, i * step:(i + 1) * step, ci * CH:(ci + 1) * CH])
            return xht, xlt

        # startup order: wk/x(0) interleaved (K matmuls start first), then
        # cos/sin (K rope), wq, wv, then attention constants.
        wkh_sb = wpool.tile([128, DT * EL], FP8, name="wkh_sb")
        wkl_sb = wpool.tile([128, DT * EL], FP8, name="wkl_sb")
        xh_cur = xpool.tile([128, DT * CH], FP8, tag="xh", name="xh0")
        xl_cur = xpool.tile([128, DT * CH], FP8, tag="xl", name="xl0")
        for (i0, i1) in [(0, 2), (2, 5), (5, 8), (8, 12), (12, 16)]:
            for t, src in ((wkh_sb, wkh), (wkl_sb, wkl)):
                nc.sync.dma_start(
                    t[:, i0 * EL:i1 * EL].rearrange("p (t e) -> p t e", t=i1 - i0),
                    src.rearrange("(t p) e -> p t e", p=128)[:, i0:i1, :])
            for t, src in ((xh_cur, xh), (xl_cur, xl)):
                nc.sync.dma_start(
                    t[:, i0 * CH:i1 * CH].rearrange("p (t c) -> p t c", t=i1 - i0),
                    src.rearrange("(t p) s -> p t s", p=128)[:, i0:i1, 0:CH])
        cos_t = cpool.tile([HD, S], BF16)
        nc.sync.dma_start(cos_t[:], cosT)
        sin_t = cpool.tile([HD, S], BF16)
        nc.sync.dma_start(sin_t[:], sinT)
        wqh_sb = load_w("wqh", wqh, DT, EL, WSPLIT)
        wql_sb = load_w("wql", wql, DT, EL, WSPLIT)
        wvh_sb = load_w("wvh", wvh, DT, EL, WSPLIT)
        wvl_sb = load_w("wvl", wvl, DT, EL, WSPLIT)
        tri_t = cpool.tile([128, 128], BF16)
        nc.sync.dma_start(tri_t[:], tri)
        ones_t = cpool.tile([128, 1], BF16)
        nc.sync.dma_start(ones_t[:], ones)
        onesT_t = cpool.tile([1, 128], F32R)
        nc.sync.dma_start(onesT_t[:], onesT)

        # ---------------- persistent K / V, z scratch ----------------
        k_c = [kvres.tile([HD, HPC * CH], BF16, name=f"k{ci}") for ci in range(NCH)]
        v_t = [kvres.tile([128, EL], BF16, name=f"v{st}") for st in range(S // 128)]
        z_shapes = [[(0, CH)]] * NCH
        z_part = {}
        z_rs = {}
        for ci in range(NCH):
            for (c0, c1) in z_shapes[ci]:
                z_part[(ci, c0)] = dram.tile([D, c1 - c0], BF16,
                                             tag=f"zp{ci}_{c0}", name=f"zp{ci}_{c0}")
                z_rs[(ci, c0)] = dram.tile([EL, c1 - c0], BF16,
                                           tag=f"zr{ci}_{c0}", name=f"zr{ci}_{c0}")

        def kq_head_mms(w_hi, w_lo, x_hi, x_lo, h, ps):
            """3-term fp8 hi/lo DoubleRow: each DR packs a k-tile pair."""
            wh = w_hi.rearrange("p (t e) -> p t e", t=DT)
            wl = w_lo.rearrange("p (t e) -> p t e", t=DT)
            xhr = x_hi.rearrange("p (t c) -> p t c", t=DT)
            xlr = x_lo.rearrange("p (t c) -> p t c", t=DT)
            hs = slice(h * HD, (h + 1) * HD)
            NP = DT // 2
            for jj in range(NP):
                ts = slice(2 * jj, 2 * jj + 2)
                for term, (wt, xt) in enumerate(
                        ((wh, xhr), (wh, xlr), (wl, xhr))):
                    nc.tensor.matmul(
                        ps[:], wt[:, ts, hs], xt[:, ts, :],
                        start=(jj == 0 and term == 0),
                        stop=(jj == NP - 1 and term == 2),
                        perf_mode=DR)

        _QKV_POOLS = [(ps_s, "s_ps"), (ps_o, "o"), (ps_mm, "ps")]
        _qkv_rot = [0]

        def qkv_psum(name):
            pool, tag = _QKV_POOLS[_qkv_rot[0] % 3]
            _qkv_rot[0] += 1
            return pool.tile([128, CH], F32, tag=tag, name=name)

        def rope_head(ci, h, x_hi, x_lo, w_hi, w_lo, out_ap, tagp):
            """One head's [HD, CH] projection + RoPE -> out_ap."""
            ps = qkv_psum(f"ps_{tagp}")
            kq_head_mms(w_hi, w_lo, x_hi, x_lo, h, ps)
            pre = rope.tile([HD, CH], BF16, tag="pre", name=f"pre_{tagp}")
            nc.scalar.copy(pre[:], ps[:])
            rot = rope.tile([HD, CH], BF16, tag="rot", name=f"rot_{tagp}")
            nc.sync.dma_start(rot[0:64, :], pre[64:128, :])
            nc.sync.dma_start(rot[64:128, :], pre[0:64, :])
            cs = cos_t[:, ci * CH:(ci + 1) * CH]
            sn = sin_t[:, ci * CH:(ci + 1) * CH]
            t1 = rope.tile([HD, CH], BF16, tag="t1", name=f"t1_{tagp}")
            t2 = rope.tile([HD, CH], BF16, tag="t2", name=f"t2_{tagp}")
            nc.vector.tensor_mul(t1[:], pre[:], cs)
            nc.vector.tensor_mul(t2[:], rot[:], sn)
            nc.vector.tensor_add(out_ap, t1[:], t2[:])

        def qkv_chunk(ci, x_hi, x_lo):
            q_sb = qpool.tile([128, HPC * CH], BF16, tag="q", name=f"q{ci}")
            # K, Q, then V: the trailing ACT evacs at attention start are V's,
            # which attention only needs at the (late) diagonal tiles — the
            # first exps never queue behind an evac.
            for h in range(HPC):
                rope_head(ci, h, x_hi, x_lo, wkh_sb, wkl_sb,
                          k_c[ci][:, h * CH:(h + 1) * CH], f"k{ci}_{h}")
            for h in range(HPC):
                rope_head(ci, h, x_hi, x_lo, wqh_sb, wql_sb,
                          q_sb[:, h * CH:(h + 1) * CH], f"q{ci}_{h}")
            wvh_r = wvh_sb.rearrange("p (t e) -> p t e", t=DT)
            wvl_r = wvl_sb.rearrange("p (t e) -> p t e", t=DT)
            xh_r = x_hi.rearrange("p (t c) -> p t c", t=DT)
            xl_r = x_lo.rearrange("p (t c) -> p t c", t=DT)
            for st in range(CH // 128):
                ps = qkv_psum(f"ps_v{ci}_{st}")
                ss = slice(st * 128, (st + 1) * 128)
                NP = DT // 2
                for jj in range(NP):
                    ts = slice(2 * jj, 2 * jj + 2)
                    for term, (xt, wt) in enumerate(
                            ((xh_r, wvh_r), (xl_r, wvh_r), (xh_r, wvl_r))):
                        nc.tensor.matmul(
                            ps[:], xt[:, ts, ss], wt[:, ts, :],
                            start=(jj == 0 and term == 0),
                            stop=(jj == NP - 1 and term == 2),
                            perf_mode=DR)
                # v psum is at QKV_SCALE; rescale to true scale on evac
                nc.scalar.mul(v_t[ci * 4 + st][:], ps[:], 1.0 / QKV_SCALE)
            return q_sb

        def attn_chunk(ci, q_sb):
            """Causal attention for query chunk ci over key chunks 0..ci.
            2-tile lookahead emission keeps PE ahead of the exp latency."""
            y_sb = ypool.tile([128, HPC * CH], BF16, tag="y", name=f"y{ci}")
            n_jt = 4 * ci + 4
            tiles = [(h, jt) for h in range(HPC) for jt in range(n_jt)]
            state = {}
            pending = []

            def emit_or(ent):
                h, jt, p, off = ent
                o_ps, r_ps = state[h]
                nc.tensor.matmul(
                    o_ps[:, off:], v_t[jt][:, h * HD:(h + 1) * HD],
                    p[:, off:], start=(jt == 0), stop=(jt == n_jt - 1))
                nc.tensor.matmul(
                    r_ps[:, off:], ones_t[:], p[:, off:],
                    start=(jt == 0), stop=(jt == n_jt - 1))
                if jt == n_jt - 1:
                    # normalize head h: y = o * (1/rowsum); the broadcast of
                    # rinv across partitions runs on the otherwise-idle
                    # gpsimd engine instead of a PE matmul.
                    rinv = rpool.tile([1, CH], F32R, tag="rinv")
                    nc.vector.reciprocal(rinv[:], r_ps[:])
                    o_sb = rpool.tile([HD, CH], F32R, tag="osb", name=f"os{ci}_{h}")
                    nc.scalar.copy(o_sb[:], o_ps[:])
                    rbc = bpool.tile([128, CH], F32R, tag="rbc", name=f"rb{ci}_{h}")
                    nc.gpsimd.partition_broadcast(rbc[:], rinv[:])
                    nc.vector.tensor_mul(
                        y_sb[:, h * CH:(h + 1) * CH], o_sb[:], rbc[:])
                    del state[h]

            for idx, (h, jt) in enumerate(tiles):
                if jt == 0:
                    o_ps = ps_o.tile([HD, CH], F32, tag="o", name=f"o{ci}_{h}")
                    r_ps = ps_r.tile([1, CH], F32, tag="r", name=f"r{ci}_{h}")
                    state[h] = (o_ps, r_ps)
                diag = jt - 4 * ci
                off = 128 * diag if diag > 0 else 0
                cj, j2 = divmod(jt, 4)
                # score tiles alternate between ps_s and the (attention-idle)
                # ps_mm pool, giving a 5-bank rotation for deeper lookahead
                spool = ps_s if idx % 2 == 0 else ps_mm
                stag = "s_ps" if idx % 2 == 0 else "ps"
                s_ps = spool.tile([128, CH], F32, tag=stag, name=f"s{ci}_{h}_{jt}")
                nc.tensor.matmul(
                    s_ps[:, off:], k_c[cj][:, h * CH + j2 * 128:h * CH + (j2 + 1) * 128],
                    q_sb[:, h * CH + off:(h + 1) * CH], start=True, stop=True)
                p = ppool.tile([128, CH], BF16, tag="p")
                nc.scalar.activation(p[:, off:], s_ps[:, off:], AF.Exp,
                                     scale=EXP_SCALE)
                if 0 <= diag:
                    nc.vector.tensor_mul(
                        p[:, off:off + 128], p[:, off:off + 128], tri_t[:])
                if len(pending) >= 5:
                    emit_or(pending.pop(0))
                pending.append((h, jt, p, off))
            for ent in pending:
                emit_or(ent)
            return y_sb

        def proj_chunk(ci, y_sb, wp_sb):
            for (c0, c1) in z_shapes[ci]:
                cw = c1 - c0
                zp = z_part[(ci, c0)]
                for eb in range(DT):
                    pool = ps_mm if eb % 2 == 0 else ps_o
                    tag = "ps" if eb % 2 == 0 else "o"
                    ps = pool.tile([128, CH], F32, tag=tag, name=f"ps_z{ci}_{eb}")
                    for ct in range(EL // 128):
                        nc.tensor.matmul(
                            ps[:, 0:cw],
                            wp_sb[:, ct * D + eb * 128:ct * D + (eb + 1) * 128],
                            y_sb[:, ct * CH + c0:ct * CH + c1],
                            start=(ct == 0), stop=(ct == EL // 128 - 1))
                    zev = ppool.tile([128, CH], BF16, tag="zev", name=f"z{ci}_{eb}")
                    if eb % 2 == 0:
                        nc.scalar.copy(zev[:, 0:cw], ps[:, 0:cw])
                    else:
                        nc.vector.tensor_copy(zev[:, 0:cw], ps[:, 0:cw])
                    nc.sync.dma_start(zp[eb * 128:(eb + 1) * 128, :], zev[:, 0:cw])
                zr = z_rs[(ci, c0)]
                nc.gpsimd.collective_compute(
                    "ReduceScatter", mybir.AluOpType.add,
                    replica_groups=[[0, 1, 2, 3], [4, 5, 6, 7]],
                    ins=[zp.opt()], outs=[zr.opt()])

        # ---------------- main loop ----------------
        wp_sb = None
        for ci in range(NCH):
            q_sb = qkv_chunk(ci, xh_cur, xl_cur)
            if ci + 1 < NCH:
                xh_cur, xl_cur = load_x(ci + 1)
            if ci == 0:
                wp_sb = load_wp("wp", wp, EL // 128, D, 2)
            y_sb = attn_chunk(ci, q_sb)
            proj_chunk(ci, y_sb, wp_sb)
        # RS cannot target an ExternalOutput; DRAM->DRAM DMAs move the
        # scattered slices into the output tensor. Emitted at the very end so
        # their RS-completion waits never head-of-line-block the SP DMA queue:
        # bounces 0..2 fire immediately, only the last waits on RS(3).
        for ci in range(NCH):
            for (c0, c1) in z_shapes[ci]:
                nc.sync.dma_start(zTc[ci * EL:(ci + 1) * EL, c0:c1],
                                  z_rs[(ci, c0)][:])
    nc.compile()
    return nc


def _tables():
    inv_freq = 1.0 / (ROPE_THETA ** (np.arange(0, HD, 2, dtype=np.float64) / HD))
    pos = np.arange(S, dtype=np.float64)
    f_half = np.outer(inv_freq, pos)                  # [64, S]
    freqs = np.concatenate([f_half, f_half], axis=0)  # [HD, S]
    emb32 = freqs.astype(np.float32)
    cos_t = np.cos(emb32) / QKV_SCALE
    sin_t = np.sin(emb32) / QKV_SCALE
    sgn = np.where(np.arange(HD) < HD // 2, -1.0, 1.0).astype(np.float32)[:, None]
    return cos_t.astype(ml_dtypes.bfloat16), (sin_t * sgn).astype(ml_dtypes.bfloat16)


def _split8(t, scale):
    """Split scale*t into fp8 hi + lo with hi = fp8(scale*t)."""
    t = np.asarray(t, np.float32) * np.float32(scale)
    hi = t.astype(ml_dtypes.float8_e4m3)
    lo = (t - hi.astype(np.float32)).astype(ml_dtypes.float8_e4m3)
    return np.ascontiguousarray(hi), np.ascontiguousarray(lo)


_NC_CACHE = {}


def _get_nc():
    if "nc" not in _NC_CACHE:
        _NC_CACHE["nc"] = _build()
    return _NC_CACHE["nc"]


def make_in_maps(x, W_attn, W_proj):
    x = np.asarray(x, dtype=np.float32)
    W_attn = np.asarray(W_attn, dtype=np.float32)
    W_proj = np.asarray(W_proj, dtype=np.float32)
    cos_t, sin_t = _tables()
    tri = np.triu(np.ones((128, 128), np.float32)).astype(ml_dtypes.bfloat16)
    ones = np.ones((128, 1), ml_dtypes.bfloat16)
    onesT = np.ones((1, 128), np.float32)
    xb = [_split8(x[b].T, SX) for b in range(B)]
    wspl = {}
    for g in range(HPC):
        wspl[g] = {
            "wq": _split8(W_attn[:, g * EL:(g + 1) * EL], SW),
            "wk": _split8(W_attn[:, D + g * EL:D + (g + 1) * EL], SW),
            "wv": _split8(W_attn[:, 2 * D + g * EL:2 * D + (g + 1) * EL], SW),
        }
    in_maps = []
    for c in range(N_CORES):
        b, g = divmod(c, HPC)
        in_maps.append({
            "xh": xb[b][0], "xl": xb[b][1],
            "wqh": wspl[g]["wq"][0], "wql": wspl[g]["wq"][1],
            "wkh": wspl[g]["wk"][0], "wkl": wspl[g]["wk"][1],
            "wvh": wspl[g]["wv"][0], "wvl": wspl[g]["wv"][1],
            "wp": np.ascontiguousarray(
                W_proj[g * EL:(g + 1) * EL, :]).astype(ml_dtypes.bfloat16),
            "cosT": cos_t, "sinT": sin_t,
            "tri": tri, "ones": ones, "onesT": onesT,
        })
    return in_maps


def assemble(results):
    out = np.empty((B, S, D), dtype=np.float32)
    for c in range(N_CORES):
        b, g = divmod(c, HPC)
        z = np.asarray(results[c]["zTc"]).astype(np.float32)   # [NCH*EL, CH]
        for ci in range(NCH):
            out[b, ci * CH:(ci + 1) * CH, g * EL:(g + 1) * EL] = \
                z[ci * EL:(ci + 1) * EL, :].T
    return out


def kernel(x, W_attn, W_proj):
    nc = _get_nc()
    in_maps = make_in_maps(x, W_attn, W_proj)
    res = bass_utils.run_bass_kernel_spmd(
        nc, in_maps, core_ids=list(range(N_CORES)), trace=False)
    return assemble(res.results)


if __name__ == "__main__":
    rng = np.random.default_rng(0)
    x = rng.standard_normal((B, S, D)).astype(np.float32)
    W_attn = (rng.standard_normal((D, 3 * D)) * D ** -0.5).astype(np.float32)
    W_proj = (rng.standard_normal((D, D)) * D ** -0.5).astype(np.float32)
    out = kernel(x, W_attn, W_proj)
    print("out", out.shape, out.dtype, np.abs(out).mean())



# revision 17
# speedup vs baseline: 1.1473x; 1.0601x over previous
"""Causal self-attention with RoPE on 8 TRN2 NeuronCores — v2.

Sharding: core c -> (batch b = c//4, head-group g = c%4; 4 heads of 128 each).
Tensor-parallel over heads x data-parallel over batch.

v2 strategy vs baseline:
  - bf16 compute throughout (inputs pre-converted on host).
  - single fused pass per 512-token chunk: x loaded once, K/Q/V computed
    together; K/V stay in SBUF (no DRAM spill), Q transient per chunk.
  - projection restructured: each core computes a FULL-WIDTH partial
    z_part = Wp[rows g].T @ y_local; a ReduceScatter(add) sums partials and
    scatters e-column slices -- replaces the 4x-more-expensive AllGather.
  - per-head RoPE chains so attention never waits on a rope DMA.
  - attention emitted with 2-tile lookahead so PE never waits on the exp.
  - batched 3-D-AP DMA loads; startup ordered so the first K matmuls can
    begin after just wk + the first slice of x.
  - last chunk's projection + ReduceScatter split in token halves to
    shorten the end-of-kernel collective tail.
"""
from contextlib import ExitStack

import numpy as np
import ml_dtypes

import concourse.bass as bass
import concourse.tile as tile
import concourse.mybir as mybir
from concourse import bacc, bass_utils

B = 2
S = 2048
D = 2048
NH, HD = 16, 128
HPC = 4                 # heads per core
EL = HPC * HD           # 512: local e-width per core
CH = 512                # token-chunk width
NCH = S // CH           # 4
DT = D // 128           # 16 d-tiles
ROPE_THETA = 10000.0
N_CORES = 8

F32 = mybir.dt.float32
F32R = mybir.dt.float32r
BF16 = mybir.dt.bfloat16
FP8 = mybir.dt.float8e4
AF = mybir.ActivationFunctionType
DR = mybir.MatmulPerfMode.DoubleRow

SX = 4.0      # fp8 scale for x
SW = 32.0     # fp8 scale for W_attn slices
QKV_SCALE = SX * SW          # q/k/v psums come out at 128x true scale
EXP_SCALE = float(HD ** -0.5)


def _build():
    nc = bacc.Bacc("TRN2", target_bir_lowering=False, debug=False,
                   enable_asserts=True, num_devices=N_CORES)
    xh = nc.dram_tensor("xh", [D, S], FP8, kind="ExternalInput").ap()
    xl = nc.dram_tensor("xl", [D, S], FP8, kind="ExternalInput").ap()
    wqh = nc.dram_tensor("wqh", [D, EL], FP8, kind="ExternalInput").ap()
    wql = nc.dram_tensor("wql", [D, EL], FP8, kind="ExternalInput").ap()
    wkh = nc.dram_tensor("wkh", [D, EL], FP8, kind="ExternalInput").ap()
    wkl = nc.dram_tensor("wkl", [D, EL], FP8, kind="ExternalInput").ap()
    wvh = nc.dram_tensor("wvh", [D, EL], FP8, kind="ExternalInput").ap()
    wvl = nc.dram_tensor("wvl", [D, EL], FP8, kind="ExternalInput").ap()
    wp = nc.dram_tensor("wp", [EL, D], BF16, kind="ExternalInput").ap()
    cosT = nc.dram_tensor("cosT", [HD, S], BF16, kind="ExternalInput").ap()
    sinT = nc.dram_tensor("sinT", [HD, S], BF16, kind="ExternalInput").ap()
    tri = nc.dram_tensor("tri", [128, 128], BF16, kind="ExternalInput").ap()
    ones = nc.dram_tensor("ones", [128, 1], BF16, kind="ExternalInput").ap()
    zTc = nc.dram_tensor("zTc", [NCH * EL, CH], BF16, kind="ExternalOutput").ap()

    with tile.TileContext(nc) as tc, \
         nc.allow_low_precision(reason="bf16 attention"), ExitStack() as ctx:
        # ---------------- pools ----------------
        cpool = ctx.enter_context(tc.tile_pool(name="const", bufs=1))
        wpool = ctx.enter_context(tc.tile_pool(name="w", bufs=1))
        xpool = ctx.enter_context(tc.tile_pool(name="x", bufs=2))
        kvres = ctx.enter_context(tc.tile_pool(name="kv", bufs=1))
        qpool = ctx.enter_context(tc.tile_pool(name="q", bufs=2))
        rope = ctx.enter_context(tc.tile_pool(name="rope", bufs=8))
        ppool = ctx.enter_context(tc.tile_pool(name="p", bufs=6))
        ypool = ctx.enter_context(tc.tile_pool(name="y", bufs=2))
        rpool = ctx.enter_context(tc.tile_pool(name="r", bufs=2))
        bpool = ctx.enter_context(tc.tile_pool(name="rbc", bufs=1))
        dram = ctx.enter_context(tc.tile_pool(name="dram", bufs=1, space="DRAM"))
        ps_mm = ctx.enter_context(tc.tile_pool(name="ps_mm", bufs=2, space="PSUM"))
        ps_s = ctx.enter_context(tc.tile_pool(name="ps_s", bufs=3, space="PSUM"))
        ps_o = ctx.enter_context(tc.tile_pool(name="ps_o", bufs=2, space="PSUM"))
        ps_r = ctx.enter_context(tc.tile_pool(name="ps_r", bufs=1, space="PSUM"))

        # ------------- weight / x loaders (split DMAs for pipelining) -------
        WSPLIT = 4            # d-tiles per weight sub-DMA

        def load_w(name, src, nt, wcols, nsub):
            t = wpool.tile([128, nt * wcols], FP8, name=name)
            step = nt // nsub
            for i in range(nsub):
                nc.sync.dma_start(
                    t[:, i * step * wcols:(i + 1) * step * wcols]
                        .rearrange("p (t e) -> p t e", t=step),
                    src.rearrange("(t p) e -> p t e", p=128)[:, i * step:(i + 1) * step, :])
            return t

        def load_wp(name, src, nt, wcols, nsub):
            t = wpool.tile([128, nt * wcols], BF16, name=name)
            step = nt // nsub
            for i in range(nsub):
                nc.sync.dma_start(
                    t[:, i * step * wcols:(i + 1) * step * wcols]
                        .rearrange("p (t e) -> p t e", t=step),
                    src.rearrange("(t p) e -> p t e", p=128)[:, i * step:(i + 1) * step, :])
            return t

        def load_x(ci):
            xht = xpool.tile([128, DT * CH], FP8, tag="xh", name=f"xh{ci}")
            xlt = xpool.tile([128, DT * CH], FP8, tag="xl", name=f"xl{ci}")
            nsub, step = 2, DT // 2
            for t, src in ((xht, xh), (xlt, xl)):
                for i in range(nsub):
                    nc.sync.dma_start(
                        t[:, i * step * CH:(i + 1) * step * CH]
                            .rearrange("p (t c) -> p t c", t=step),
                        src.rearrange("(t p) s -> p t s", p=128)
                          [:, i * step:(i + 1) * step, ci * CH:(ci + 1) * CH])
            return xht, xlt

        # startup order: wk/x(0) interleaved (K matmuls start first), then
        # cos/sin (K rope), wq, wv, then attention constants.
        wkh_sb = wpool.tile([128, DT * EL], FP8, name="wkh_sb")
        wkl_sb = wpool.tile([128, DT * EL], FP8, name="wkl_sb")
        xh_cur = xpool.tile([128, DT * CH], FP8, tag="xh", name="xh0")
        xl_cur = xpool.tile([128, DT * CH], FP8, tag="xl", name="xl0")
        for (i0, i1) in [(0, 2), (2, 5), (5, 8), (8, 12), (12, 16)]:
            for t, src in ((wkh_sb, wkh), (wkl_sb, wkl)):
                nc.sync.dma_start(
                    t[:, i0 * EL:i1 * EL].rearrange("p (t e) -> p t e", t=i1 - i0),
                    src.rearrange("(t p) e -> p t e", p=128)[:, i0:i1, :])
            for t, src in ((xh_cur, xh), (xl_cur, xl)):
                nc.sync.dma_start(
                    t[:, i0 * CH:i1 * CH].rearrange("p (t c) -> p t c", t=i1 - i0),
                    src.rearrange("(t p) s -> p t s", p=128)[:, i0:i1, 0:CH])
        cos_t = cpool.tile([HD, S], BF16)
        nc.sync.dma_start(cos_t[:], cosT)
        sin_t = cpool.tile([HD, S], BF16)
        nc.sync.dma_start(sin_t[:], sinT)
        wqh_sb = load_w("wqh", wqh, DT, EL, WSPLIT)
        wql_sb = load_w("wql", wql, DT, EL, WSPLIT)
        wvh_sb = load_w("wvh", wvh, DT, EL, WSPLIT)
        wvl_sb = load_w("wvl", wvl, DT, EL, WSPLIT)
        tri_t = cpool.tile([128, 128], BF16)
        nc.sync.dma_start(tri_t[:], tri)
        ones_t = cpool.tile([128, 1], BF16)
        nc.sync.dma_start(ones_t[:], ones)

        # ---------------- persistent K / V, z scratch ----------------
        k_c = [kvres.tile([HD, HPC * CH], BF16, name=f"k{ci}") for ci in range(NCH)]
        v_t = [kvres.tile([128, EL], BF16, name=f"v{st}") for st in range(S // 128)]
        z_shapes = [[(0, CH)]] * NCH
        z_part = {}
        z_rs = {}
        for ci in range(NCH):
            for (c0, c1) in z_shapes[ci]:
                z_part[(ci, c0)] = dram.tile([D, c1 - c0], BF16,
                                             tag=f"zp{ci}_{c0}", name=f"zp{ci}_{c0}")
                # padded so the RS writes a strided row-major region
                z_rs[(ci, c0)] = dram.tile([EL, c1 - c0 + 64], BF16,
                                           tag=f"zr{ci}_{c0}", name=f"zr{ci}_{c0}")

        def kq_head_mms(w_hi, w_lo, x_hi, x_lo, h, ps):
            """3-term fp8 hi/lo DoubleRow: each DR packs a k-tile pair."""
            wh = w_hi.rearrange("p (t e) -> p t e", t=DT)
            wl = w_lo.rearrange("p (t e) -> p t e", t=DT)
            xhr = x_hi.rearrange("p (t c) -> p t c", t=DT)
            xlr = x_lo.rearrange("p (t c) -> p t c", t=DT)
            hs = slice(h * HD, (h + 1) * HD)
            NP = DT // 2
            for jj in range(NP):
                ts = slice(2 * jj, 2 * jj + 2)
                for term, (wt, xt) in enumerate(
                        ((wh, xhr), (wh, xlr), (wl, xhr))):
                    nc.tensor.matmul(
                        ps[:], wt[:, ts, hs], xt[:, ts, :],
                        start=(jj == 0 and term == 0),
                        stop=(jj == NP - 1 and term == 2),
                        perf_mode=DR)

        _QKV_POOLS = [(ps_s, "s_ps"), (ps_o, "o"), (ps_mm, "ps")]
        _qkv_rot = [0]

        def qkv_psum(name):
            pool, tag = _QKV_POOLS[_qkv_rot[0] % 3]
            _qkv_rot[0] += 1
            return pool.tile([128, CH], F32, tag=tag, name=name)

        def rope_head(ci, h, x_hi, x_lo, w_hi, w_lo, out_ap, tagp):
            """One head's [HD, CH] projection + RoPE -> out_ap."""
            ps = qkv_psum(f"ps_{tagp}")
            kq_head_mms(w_hi, w_lo, x_hi, x_lo, h, ps)
            pre = rope.tile([HD, CH], BF16, tag="pre", name=f"pre_{tagp}")
            nc.scalar.copy(pre[:], ps[:])
            rot = rope.tile([HD, CH], BF16, tag="rot", name=f"rot_{tagp}")
            nc.sync.dma_start(rot[0:64, :], pre[64:128, :])
            nc.sync.dma_start(rot[64:128, :], pre[0:64, :])
            cs = cos_t[:, ci * CH:(ci + 1) * CH]
            sn = sin_t[:, ci * CH:(ci + 1) * CH]
            t1 = rope.tile([HD, CH], BF16, tag="t1", name=f"t1_{tagp}")
            t2 = rope.tile([HD, CH], BF16, tag="t2", name=f"t2_{tagp}")
            nc.vector.tensor_mul(t1[:], pre[:], cs)
            nc.vector.tensor_mul(t2[:], rot[:], sn)
            nc.vector.tensor_add(out_ap, t1[:], t2[:])

        def qkv_chunk(ci, x_hi, x_lo):
            q_sb = qpool.tile([128, HPC * CH], BF16, tag="q", name=f"q{ci}")
            # K, Q, then V: the trailing ACT evacs at attention start are V's,
            # which attention only needs at the (late) diagonal tiles — the
            # first exps never queue behind an evac.
            for h in range(HPC):
                rope_head(ci, h, x_hi, x_lo, wkh_sb, wkl_sb,
                          k_c[ci][:, h * CH:(h + 1) * CH], f"k{ci}_{h}")
            for h in range(HPC):
                rope_head(ci, h, x_hi, x_lo, wqh_sb, wql_sb,
                          q_sb[:, h * CH:(h + 1) * CH], f"q{ci}_{h}")
            wvh_r = wvh_sb.rearrange("p (t e) -> p t e", t=DT)
            wvl_r = wvl_sb.rearrange("p (t e) -> p t e", t=DT)
            xh_r = x_hi.rearrange("p (t c) -> p t c", t=DT)
            xl_r = x_lo.rearrange("p (t c) -> p t c", t=DT)
            for st in range(CH // 128):
                ps = qkv_psum(f"ps_v{ci}_{st}")
                ss = slice(st * 128, (st + 1) * 128)
                NP = DT // 2
                for jj in range(NP):
                    ts = slice(2 * jj, 2 * jj + 2)
                    for term, (xt, wt) in enumerate(
                            ((xh_r, wvh_r), (xl_r, wvh_r), (xh_r, wvl_r))):
                        nc.tensor.matmul(
                            ps[:], xt[:, ts, ss], wt[:, ts, :],
                            start=(jj == 0 and term == 0),
                            stop=(jj == NP - 1 and term == 2),
                            perf_mode=DR)
                # v psum is at QKV_SCALE; rescale to true scale on evac
                nc.scalar.mul(v_t[ci * 4 + st][:], ps[:], 1.0 / QKV_SCALE)
            return q_sb

        def attn_chunk(ci, q_sb):
            """Causal attention for query chunk ci over key chunks 0..ci.
            2-tile lookahead emission keeps PE ahead of the exp latency."""
            y_sb = ypool.tile([128, HPC * CH], BF16, tag="y", name=f"y{ci}")
            n_jt = 4 * ci + 4
            tiles = [(h, jt) for h in range(HPC) for jt in range(n_jt)]
            state = {}
            pending = []

            def emit_or(ent):
                h, jt, p, off = ent
                o_ps, r_ps = state[h]
                nc.tensor.matmul(
                    o_ps[:, off:], v_t[jt][:, h * HD:(h + 1) * HD],
                    p[:, off:], start=(jt == 0), stop=(jt == n_jt - 1))
                # rowsum with p as the stationary operand: out is [128q, 1]
                # per 128-query slice, accumulated across key tiles in psum.
                for qs in range(off // 128, 4):
                    nc.tensor.matmul(
                        r_ps[:, qs:qs + 1], p[:, qs * 128:(qs + 1) * 128],
                        ones_t[:], start=(jt == 0 and qs == 0),
                        stop=(jt == n_jt - 1 and qs == 3))
                if jt == n_jt - 1:
                    # normalize head h: y = o * (1/rowsum). rinv comes out
                    # with queries on partitions; a flattening DMA rebuilds
                    # the [1, CH] row for the partition broadcast.
                    rinv = rpool.tile([128, 4], BF16, tag="rinv")
                    nc.vector.reciprocal(rinv[:], r_ps[:, 0:4])
                    # flatten [128,4] -> [1,512] in source order; rrow/rbc end
                    # up (q,j)-interleaved, deinterleaved by the read AP below
                    rrow = rpool.tile([1, CH], BF16, tag="rrow",
                                      name=f"rr{ci}_{h}")
                    nc.sync.dma_start(rrow[:], rinv[:])
                    rbc = bpool.tile([128, CH], BF16, tag="rbc", name=f"rb{ci}_{h}")
                    nc.gpsimd.partition_broadcast(rbc[:], rrow[:])
                    nc.vector.tensor_mul(
                        y_sb[:, h * CH:(h + 1) * CH], o_ps[:],
                        rbc.rearrange("p (q j) -> p j q", j=4))
                    del state[h]

            for idx, (h, jt) in enumerate(tiles):
                if jt == 0:
                    o_ps = ps_o.tile([HD, CH], F32, tag="o", name=f"o{ci}_{h}")
                    r_ps = ps_r.tile([128, CH], F32, tag="r", name=f"r{ci}_{h}")
                    state[h] = (o_ps, r_ps)
                diag = jt - 4 * ci
                off = 128 * diag if diag > 0 else 0
                cj, j2 = divmod(jt, 4)
                # score tiles alternate between ps_s and the (attention-idle)
                # ps_mm pool, giving a 5-bank rotation for deeper lookahead
                spool = ps_s if idx % 2 == 0 else ps_mm
                stag = "s_ps" if idx % 2 == 0 else "ps"
                s_ps = spool.tile([128, CH], F32, tag=stag, name=f"s{ci}_{h}_{jt}")
                nc.tensor.matmul(
                    s_ps[:, off:], k_c[cj][:, h * CH + j2 * 128:h * CH + (j2 + 1) * 128],
                    q_sb[:, h * CH + off:(h + 1) * CH], start=True, stop=True)
                p = ppool.tile([128, CH], BF16, tag="p")
                nc.scalar.activation(p[:, off:], s_ps[:, off:], AF.Exp,
                                     scale=EXP_SCALE)
                if 0 <= diag:
                    nc.vector.tensor_mul(
                        p[:, off:off + 128], p[:, off:off + 128], tri_t[:])
                if len(pending) >= 5:
                    emit_or(pending.pop(0))
                pending.append((h, jt, p, off))
            for ent in pending:
                emit_or(ent)
            return y_sb

        def proj_chunk(ci, y_sb, wp_sb):
            for (c0, c1) in z_shapes[ci]:
                cw = c1 - c0
                zp = z_part[(ci, c0)]
                for eb in range(DT):
                    pool = ps_mm if eb % 2 == 0 else ps_o
                    tag = "ps" if eb % 2 == 0 else "o"
                    ps = pool.tile([128, CH], F32, tag=tag, name=f"ps_z{ci}_{eb}")
                    for ct in range(EL // 128):
                        nc.tensor.matmul(
                            ps[:, 0:cw],
                            wp_sb[:, ct * D + eb * 128:ct * D + (eb + 1) * 128],
                            y_sb[:, ct * CH + c0:ct * CH + c1],
                            start=(ct == 0), stop=(ct == EL // 128 - 1))
                    zev = ppool.tile([128, CH], BF16, tag="zev", name=f"z{ci}_{eb}")
                    if eb % 2 == 0:
                        nc.scalar.copy(zev[:, 0:cw], ps[:, 0:cw])
                    else:
                        nc.vector.tensor_copy(zev[:, 0:cw], ps[:, 0:cw])
                    nc.sync.dma_start(zp[eb * 128:(eb + 1) * 128, :], zev[:, 0:cw])
                zr = z_rs[(ci, c0)]
                nc.gpsimd.collective_compute(
                    "ReduceScatter", mybir.AluOpType.add,
                    replica_groups=[[0, 1, 2, 3], [4, 5, 6, 7]],
                    ins=[zp.opt()], outs=[zr[:, 0:cw]])

        # ---------------- main loop ----------------
        wp_sb = None
        for ci in range(NCH):
            q_sb = qkv_chunk(ci, xh_cur, xl_cur)
            if ci + 1 < NCH:
                xh_cur, xl_cur = load_x(ci + 1)
            if ci == 0:
                wp_sb = load_wp("wp", wp, EL // 128, D, 2)
            y_sb = attn_chunk(ci, q_sb)
            proj_chunk(ci, y_sb, wp_sb)
        # RS cannot target an ExternalOutput; DRAM->DRAM DMAs move the
        # scattered slices into the output tensor. Emitted at the very end so
        # their RS-completion waits never head-of-line-block the SP DMA queue:
        # bounces 0..2 fire immediately, only the last waits on RS(3).
        for ci in range(NCH):
            for (c0, c1) in z_shapes[ci]:
                nc.sync.dma_start(zTc[ci * EL:(ci + 1) * EL, c0:c1],
                                  z_rs[(ci, c0)][:, 0:c1 - c0])
    nc.compile()
    return nc


def _tables():
    inv_freq = 1.0 / (ROPE_THETA ** (np.arange(0, HD, 2, dtype=np.float64) / HD))
    pos = np.arange(S, dtype=np.float64)
    f_half = np.outer(inv_freq, pos)                  # [64, S]
    freqs = np.concatenate([f_half, f_half], axis=0)  # [HD, S]
    emb32 = freqs.astype(np.float32)
    cos_t = np.cos(emb32) / QKV_SCALE
    sin_t = np.sin(emb32) / QKV_SCALE
    sgn = np.where(np.arange(HD) < HD // 2, -1.0, 1.0).astype(np.float32)[:, None]
    return cos_t.astype(ml_dtypes.bfloat16), (sin_t * sgn).astype(ml_dtypes.bfloat16)


def _split8(t, scale):
    """Split scale*t into fp8 hi + lo with hi = fp8(scale*t)."""
    t = np.asarray(t, np.float32) * np.float32(scale)
    hi = t.astype(ml_dtypes.float8_e4m3)
    lo = (t - hi.astype(np.float32)).astype(ml_dtypes.float8_e4m3)
    return np.ascontiguousarray(hi), np.ascontiguousarray(lo)


_NC_CACHE = {}


def _get_nc():
    if "nc" not in _NC_CACHE:
        _NC_CACHE["nc"] = _build()
    return _NC_CACHE["nc"]


def make_in_maps(x, W_attn, W_proj):
    x = np.asarray(x, dtype=np.float32)
    W_attn = np.asarray(W_attn, dtype=np.float32)
    W_proj = np.asarray(W_proj, dtype=np.float32)
    cos_t, sin_t = _tables()
    tri = np.triu(np.ones((128, 128), np.float32)).astype(ml_dtypes.bfloat16)
    ones = np.ones((128, 1), ml_dtypes.bfloat16)
    xb = [_split8(x[b].T, SX) for b in range(B)]
    wspl = {}
    for g in range(HPC):
        wspl[g] = {
            "wq": _split8(W_attn[:, g * EL:(g + 1) * EL], SW),
            "wk": _split8(W_attn[:, D + g * EL:D + (g + 1) * EL], SW),
            "wv": _split8(W_attn[:, 2 * D + g * EL:2 * D + (g + 1) * EL], SW),
        }
    in_maps = []
    for c in range(N_CORES):
        b, g = divmod(c, HPC)
        in_maps.append({
            "xh": xb[b][0], "xl": xb[b][1],
            "wqh": wspl[g]["wq"][0], "wql": wspl[g]["wq"][1],
            "wkh": wspl[g]["wk"][0], "wkl": wspl[g]["wk"][1],
            "wvh": wspl[g]["wv"][0], "wvl": wspl[g]["wv"][1],
            "wp": np.ascontiguousarray(
                W_proj[g * EL:(g + 1) * EL, :]).astype(ml_dtypes.bfloat16),
            "cosT": cos_t, "sinT": sin_t,
            "tri": tri, "ones": ones,
        })
    return in_maps


def assemble(results):
    out = np.empty((B, S, D), dtype=np.float32)
    for c in range(N_CORES):
        b, g = divmod(c, HPC)
        z = np.asarray(results[c]["zTc"]).astype(np.float32)   # [NCH*EL, CH]
        for ci in range(NCH):
            out[b, ci * CH:(ci + 1) * CH, g * EL:(g + 1) * EL] = \
                z[ci * EL:(ci + 1) * EL, :].T
    return out


def kernel(x, W_attn, W_proj):
    nc = _get_nc()
    in_maps = make_in_maps(x, W_attn, W_proj)
    res = bass_utils.run_bass_kernel_spmd(
        nc, in_maps, core_ids=list(range(N_CORES)), trace=False)
    return assemble(res.results)


if __name__ == "__main__":
    rng = np.random.default_rng(0)
    x = rng.standard_normal((B, S, D)).astype(np.float32)
    W_attn = (rng.standard_normal((D, 3 * D)) * D ** -0.5).astype(np.float32)
    W_proj = (rng.standard_normal((D, D)) * D ** -0.5).astype(np.float32)
    out = kernel(x, W_attn, W_proj)
    print("out", out.shape, out.dtype, np.abs(out).mean())



# revision 22
# speedup vs baseline: 1.1490x; 1.0015x over previous
"""Causal self-attention with RoPE on 8 TRN2 NeuronCores — v2.

Sharding: core c -> (batch b = c//4, head-group g = c%4; 4 heads of 128 each).
Tensor-parallel over heads x data-parallel over batch.

v2 strategy vs baseline:
  - bf16 compute throughout (inputs pre-converted on host).
  - single fused pass per 512-token chunk: x loaded once, K/Q/V computed
    together; K/V stay in SBUF (no DRAM spill), Q transient per chunk.
  - projection restructured: each core computes a FULL-WIDTH partial
    z_part = Wp[rows g].T @ y_local; a ReduceScatter(add) sums partials and
    scatters e-column slices -- replaces the 4x-more-expensive AllGather.
  - per-head RoPE chains so attention never waits on a rope DMA.
  - attention emitted with 2-tile lookahead so PE never waits on the exp.
  - batched 3-D-AP DMA loads; startup ordered so the first K matmuls can
    begin after just wk + the first slice of x.
  - last chunk's projection + ReduceScatter split in token halves to
    shorten the end-of-kernel collective tail.
"""
from contextlib import ExitStack

import numpy as np
import ml_dtypes

import concourse.bass as bass
import concourse.tile as tile
import concourse.mybir as mybir
from concourse import bacc, bass_utils

B = 2
S = 2048
D = 2048
NH, HD = 16, 128
HPC = 4                 # heads per core
EL = HPC * HD           # 512: local e-width per core
CH = 512                # token-chunk width
NCH = S // CH           # 4
DT = D // 128           # 16 d-tiles
ROPE_THETA = 10000.0
N_CORES = 8

F32 = mybir.dt.float32
F32R = mybir.dt.float32r
BF16 = mybir.dt.bfloat16
FP8 = mybir.dt.float8e4
AF = mybir.ActivationFunctionType
DR = mybir.MatmulPerfMode.DoubleRow

SX = 4.0      # fp8 scale for x
SW = 32.0     # fp8 scale for W_attn slices
QKV_SCALE = SX * SW          # q/k/v psums come out at 128x true scale
EXP_SCALE = float(HD ** -0.5)


def _build():
    nc = bacc.Bacc("TRN2", target_bir_lowering=False, debug=False,
                   enable_asserts=True, num_devices=N_CORES)
    xh = nc.dram_tensor("xh", [D, S], FP8, kind="ExternalInput").ap()
    xl = nc.dram_tensor("xl", [D, S], FP8, kind="ExternalInput").ap()
    wqh = nc.dram_tensor("wqh", [D, EL], FP8, kind="ExternalInput").ap()
    wql = nc.dram_tensor("wql", [D, EL], FP8, kind="ExternalInput").ap()
    wkh = nc.dram_tensor("wkh", [D, EL], FP8, kind="ExternalInput").ap()
    wkl = nc.dram_tensor("wkl", [D, EL], FP8, kind="ExternalInput").ap()
    wvh = nc.dram_tensor("wvh", [D, EL], FP8, kind="ExternalInput").ap()
    wvl = nc.dram_tensor("wvl", [D, EL], FP8, kind="ExternalInput").ap()
    wp = nc.dram_tensor("wp", [EL, D], BF16, kind="ExternalInput").ap()
    cosT = nc.dram_tensor("cosT", [HD, S], BF16, kind="ExternalInput").ap()
    sinT = nc.dram_tensor("sinT", [HD, S], BF16, kind="ExternalInput").ap()
    tri = nc.dram_tensor("tri", [128, 128], BF16, kind="ExternalInput").ap()
    ones = nc.dram_tensor("ones", [128, 1], BF16, kind="ExternalInput").ap()
    zTc = nc.dram_tensor("zTc", [NCH * EL, CH], BF16, kind="ExternalOutput").ap()

    with tile.TileContext(nc) as tc, \
         nc.allow_low_precision(reason="bf16 attention"), ExitStack() as ctx:
        # ---------------- pools ----------------
        cpool = ctx.enter_context(tc.tile_pool(name="const", bufs=1))
        wpool = ctx.enter_context(tc.tile_pool(name="w", bufs=1))
        xpool = ctx.enter_context(tc.tile_pool(name="x", bufs=2))
        kvres = ctx.enter_context(tc.tile_pool(name="kv", bufs=1))
        qpool = ctx.enter_context(tc.tile_pool(name="q", bufs=2))
        rope = ctx.enter_context(tc.tile_pool(name="rope", bufs=8))
        ppool = ctx.enter_context(tc.tile_pool(name="p", bufs=6))
        ypool = ctx.enter_context(tc.tile_pool(name="y", bufs=2))
        rpool = ctx.enter_context(tc.tile_pool(name="r", bufs=2))
        bpool = ctx.enter_context(tc.tile_pool(name="rbc", bufs=1))
        dram = ctx.enter_context(tc.tile_pool(name="dram", bufs=1, space="DRAM"))
        ps_mm = ctx.enter_context(tc.tile_pool(name="ps_mm", bufs=2, space="PSUM"))
        ps_s = ctx.enter_context(tc.tile_pool(name="ps_s", bufs=3, space="PSUM"))
        ps_o = ctx.enter_context(tc.tile_pool(name="ps_o", bufs=2, space="PSUM"))
        ps_r = ctx.enter_context(tc.tile_pool(name="ps_r", bufs=1, space="PSUM"))

        # ------------- weight / x loaders (split DMAs for pipelining) -------
        WSPLIT = 4            # d-tiles per weight sub-DMA

        def load_w(name, src, nt, wcols, nsub):
            t = wpool.tile([128, nt * wcols], FP8, name=name)
            step = nt // nsub
            for i in range(nsub):
                nc.sync.dma_start(
                    t[:, i * step * wcols:(i + 1) * step * wcols]
                        .rearrange("p (t e) -> p t e", t=step),
                    src.rearrange("(t p) e -> p t e", p=128)[:, i * step:(i + 1) * step, :])
            return t

        def load_wp(name, src, nt, wcols, nsub):
            t = wpool.tile([128, nt * wcols], BF16, name=name)
            step = nt // nsub
            for i in range(nsub):
                nc.sync.dma_start(
                    t[:, i * step * wcols:(i + 1) * step * wcols]
                        .rearrange("p (t e) -> p t e", t=step),
                    src.rearrange("(t p) e -> p t e", p=128)[:, i * step:(i + 1) * step, :])
            return t

        def load_x(ci):
            xht = xpool.tile([128, DT * CH], FP8, tag="xh", name=f"xh{ci}")
            xlt = xpool.tile([128, DT * CH], FP8, tag="xl", name=f"xl{ci}")
            nsub, step = 2, DT // 2
            for t, src in ((xht, xh), (xlt, xl)):
                for i in range(nsub):
                    nc.sync.dma_start(
                        t[:, i * step * CH:(i + 1) * step * CH]
                            .rearrange("p (t c) -> p t c", t=step),
                        src.rearrange("(t p) s -> p t s", p=128)
                          [:, i * step:(i + 1) * step, ci * CH:(ci + 1) * CH])
            return xht, xlt

        # startup order: wk/x(0) interleaved (K matmuls start first), then
        # cos/sin (K rope), wq, wv, then attention constants.
        wkh_sb = wpool.tile([128, DT * EL], FP8, name="wkh_sb")
        wkl_sb = wpool.tile([128, DT * EL], FP8, name="wkl_sb")
        xh_cur = xpool.tile([128, DT * CH], FP8, tag="xh", name="xh0")
        xl_cur = xpool.tile([128, DT * CH], FP8, tag="xl", name="xl0")
        for (i0, i1) in [(0, 2), (2, 5), (5, 8), (8, 12), (12, 16)]:
            nc.sync.dma_start(
                wkh_sb[:, i0 * EL:i1 * EL].rearrange("p (t e) -> p t e", t=i1 - i0),
                wkh.rearrange("(t p) e -> p t e", p=128)[:, i0:i1, :])
            nc.sync.dma_start(
                xh_cur[:, i0 * CH:i1 * CH].rearrange("p (t c) -> p t c", t=i1 - i0),
                xh.rearrange("(t p) s -> p t s", p=128)[:, i0:i1, 0:CH])
            nc.sync.dma_start(
                wkl_sb[:, i0 * EL:i1 * EL].rearrange("p (t e) -> p t e", t=i1 - i0),
                wkl.rearrange("(t p) e -> p t e", p=128)[:, i0:i1, :])
            nc.sync.dma_start(
                xl_cur[:, i0 * CH:i1 * CH].rearrange("p (t c) -> p t c", t=i1 - i0),
                xl.rearrange("(t p) s -> p t s", p=128)[:, i0:i1, 0:CH])
        cos_t = cpool.tile([HD, S], BF16)
        nc.sync.dma_start(cos_t[:], cosT)
        sin_t = cpool.tile([HD, S], BF16)
        nc.sync.dma_start(sin_t[:], sinT)
        wqh_sb = load_w("wqh", wqh, DT, EL, WSPLIT)
        wql_sb = load_w("wql", wql, DT, EL, WSPLIT)
        wvh_sb = load_w("wvh", wvh, DT, EL, WSPLIT)
        wvl_sb = load_w("wvl", wvl, DT, EL, WSPLIT)
        tri_t = cpool.tile([128, 128], BF16)
        nc.sync.dma_start(tri_t[:], tri)
        ones_t = cpool.tile([128, 1], BF16)
        nc.sync.dma_start(ones_t[:], ones)

        # ---------------- persistent K / V, z scratch ----------------
        k_c = [kvres.tile([HD, HPC * CH], BF16, name=f"k{ci}") for ci in range(NCH)]
        v_t = [kvres.tile([128, EL], BF16, name=f"v{st}") for st in range(S // 128)]
        z_shapes = [[(0, CH)]] * NCH
        z_part = {}
        z_rs = {}
        for ci in range(NCH):
            for (c0, c1) in z_shapes[ci]:
                z_part[(ci, c0)] = dram.tile([D, c1 - c0], BF16,
                                             tag=f"zp{ci}_{c0}", name=f"zp{ci}_{c0}")
                # padded so the RS writes a strided row-major region
                z_rs[(ci, c0)] = dram.tile([EL, c1 - c0 + 64], BF16,
                                           tag=f"zr{ci}_{c0}", name=f"zr{ci}_{c0}")

        def kq_head_mms(w_hi, w_lo, x_hi, x_lo, h, ps):
            """3-term fp8 hi/lo DoubleRow: each DR packs a k-tile pair."""
            wh = w_hi.rearrange("p (t e) -> p t e", t=DT)
            wl = w_lo.rearrange("p (t e) -> p t e", t=DT)
            xhr = x_hi.rearrange("p (t c) -> p t c", t=DT)
            xlr = x_lo.rearrange("p (t c) -> p t c", t=DT)
            hs = slice(h * HD, (h + 1) * HD)
            NP = DT // 2
            # term-major: all hi*hi first, so matmuls start as soon as the
            # hi tensors' first slices land, while lo tensors still stream in
            for term, (wt, xt) in enumerate(((wh, xhr), (wh, xlr), (wl, xhr))):
                for jj in range(NP):
                    ts = slice(2 * jj, 2 * jj + 2)
                    nc.tensor.matmul(
                        ps[:], wt[:, ts, hs], xt[:, ts, :],
                        start=(jj == 0 and term == 0),
                        stop=(jj == NP - 1 and term == 2),
                        perf_mode=DR)

        _QKV_POOLS = [(ps_s, "s_ps"), (ps_o, "o"), (ps_mm, "ps")]
        _qkv_rot = [0]

        def qkv_psum(name):
            pool, tag = _QKV_POOLS[_qkv_rot[0] % 3]
            _qkv_rot[0] += 1
            return pool.tile([128, CH], F32, tag=tag, name=name)

        def rope_head(ci, h, x_hi, x_lo, w_hi, w_lo, out_ap, tagp):
            """One head's [HD, CH] projection + RoPE -> out_ap."""
            ps = qkv_psum(f"ps_{tagp}")
            kq_head_mms(w_hi, w_lo, x_hi, x_lo, h, ps)
            pre = rope.tile([HD, CH], BF16, tag="pre", name=f"pre_{tagp}")
            nc.gpsimd.tensor_copy(pre[:], ps[:])
            rot = rope.tile([HD, CH], BF16, tag="rot", name=f"rot_{tagp}")
            nc.sync.dma_start(rot[0:64, :], pre[64:128, :])
            nc.sync.dma_start(rot[64:128, :], pre[0:64, :])
            cs = cos_t[:, ci * CH:(ci + 1) * CH]
            sn = sin_t[:, ci * CH:(ci + 1) * CH]
            t1 = rope.tile([HD, CH], BF16, tag="t1", name=f"t1_{tagp}")
            t2 = rope.tile([HD, CH], BF16, tag="t2", name=f"t2_{tagp}")
            nc.vector.tensor_mul(t1[:], pre[:], cs)
            nc.vector.tensor_mul(t2[:], rot[:], sn)
            nc.vector.tensor_add(out_ap, t1[:], t2[:])

        def qkv_chunk(ci, x_hi, x_lo):
            q_sb = qpool.tile([128, HPC * CH], BF16, tag="q", name=f"q{ci}")
            # K, Q, then V: the trailing ACT evacs at attention start are V's,
            # which attention only needs at the (late) diagonal tiles — the
            # first exps never queue behind an evac.
            for h in range(HPC):
                rope_head(ci, h, x_hi, x_lo, wkh_sb, wkl_sb,
                          k_c[ci][:, h * CH:(h + 1) * CH], f"k{ci}_{h}")
            for h in range(HPC):
                rope_head(ci, h, x_hi, x_lo, wqh_sb, wql_sb,
                          q_sb[:, h * CH:(h + 1) * CH], f"q{ci}_{h}")
            wvh_r = wvh_sb.rearrange("p (t e) -> p t e", t=DT)
            wvl_r = wvl_sb.rearrange("p (t e) -> p t e", t=DT)
            xh_r = x_hi.rearrange("p (t c) -> p t c", t=DT)
            xl_r = x_lo.rearrange("p (t c) -> p t c", t=DT)
            for st in range(CH // 128):
                ps = qkv_psum(f"ps_v{ci}_{st}")
                ss = slice(st * 128, (st + 1) * 128)
                NP = DT // 2
                for jj in range(NP):
                    ts = slice(2 * jj, 2 * jj + 2)
                    for term, (xt, wt) in enumerate(
                            ((xh_r, wvh_r), (xl_r, wvh_r), (xh_r, wvl_r))):
                        nc.tensor.matmul(
                            ps[:], xt[:, ts, ss], wt[:, ts, :],
                            start=(jj == 0 and term == 0),
                            stop=(jj == NP - 1 and term == 2),
                            perf_mode=DR)
                # v psum is at QKV_SCALE; rescale to true scale on evac
                nc.gpsimd.tensor_scalar_mul(v_t[ci * 4 + st][:], ps[:],
                                            1.0 / QKV_SCALE)
            return q_sb

        def attn_chunk(ci, q_sb):
            """Causal attention for query chunk ci over key chunks 0..ci.
            2-tile lookahead emission keeps PE ahead of the exp latency."""
            y_sb = ypool.tile([128, HPC * CH], BF16, tag="y", name=f"y{ci}")
            n_jt = 4 * ci + 4
            tiles = [(h, jt) for h in range(HPC) for jt in range(n_jt)]
            state = {}
            pending = []

            def emit_or(ent):
                h, jt, p, off = ent
                o_ps, r_ps = state[h]
                nc.tensor.matmul(
                    o_ps[:, off:], v_t[jt][:, h * HD:(h + 1) * HD],
                    p[:, off:], start=(jt == 0), stop=(jt == n_jt - 1))
                # rowsum with p as the stationary operand: out is [128q, 1]
                # per 128-query slice, accumulated across key tiles in psum.
                for qs in range(off // 128, 4):
                    nc.tensor.matmul(
                        r_ps[:, qs:qs + 1], p[:, qs * 128:(qs + 1) * 128],
                        ones_t[:], start=(jt == 0 and qs == 0),
                        stop=(jt == n_jt - 1 and qs == 3))
                if jt == n_jt - 1:
                    # normalize head h: y = o * (1/rowsum). rinv comes out
                    # with queries on partitions; a flattening DMA rebuilds
                    # the [1, CH] row for the partition broadcast.
                    rinv = rpool.tile([128, 4], BF16, tag="rinv")
                    nc.vector.reciprocal(rinv[:], r_ps[:, 0:4])
                    # flatten [128,4] -> [1,512] in source order; rrow/rbc end
                    # up (q,j)-interleaved, deinterleaved by the read AP below
                    rrow = rpool.tile([1, CH], BF16, tag="rrow",
                                      name=f"rr{ci}_{h}")
                    nc.sync.dma_start(rrow[:], rinv[:])
                    rbc = bpool.tile([128, CH], BF16, tag="rbc", name=f"rb{ci}_{h}")
                    nc.gpsimd.partition_broadcast(rbc[:], rrow[:])
                    nc.vector.tensor_mul(
                        y_sb[:, h * CH:(h + 1) * CH], o_ps[:],
                        rbc.rearrange("p (q j) -> p j q", j=4))
                    del state[h]

            for idx, (h, jt) in enumerate(tiles):
                if jt == 0:
                    o_ps = ps_o.tile([HD, CH], F32, tag="o", name=f"o{ci}_{h}")
                    r_ps = ps_r.tile([128, CH], F32, tag="r", name=f"r{ci}_{h}")
                    state[h] = (o_ps, r_ps)
                diag = jt - 4 * ci
                off = 128 * diag if diag > 0 else 0
                cj, j2 = divmod(jt, 4)
                # score tiles alternate between ps_s and the (attention-idle)
                # ps_mm pool, giving a 5-bank rotation for deeper lookahead
                spool = ps_s if idx % 2 == 0 else ps_mm
                stag = "s_ps" if idx % 2 == 0 else "ps"
                s_ps = spool.tile([128, CH], F32, tag=stag, name=f"s{ci}_{h}_{jt}")
                nc.tensor.matmul(
                    s_ps[:, off:], k_c[cj][:, h * CH + j2 * 128:h * CH + (j2 + 1) * 128],
                    q_sb[:, h * CH + off:(h + 1) * CH], start=True, stop=True)
                p = ppool.tile([128, CH], BF16, tag="p")
                nc.scalar.activation(p[:, off:], s_ps[:, off:], AF.Exp,
                                     scale=EXP_SCALE)
                if 0 <= diag:
                    nc.vector.tensor_mul(
                        p[:, off:off + 128], p[:, off:off + 128], tri_t[:])
                if len(pending) >= 5:
                    emit_or(pending.pop(0))
                pending.append((h, jt, p, off))
            for ent in pending:
                emit_or(ent)
            return y_sb

        def proj_chunk(ci, y_sb, wp_sb):
            for (c0, c1) in z_shapes[ci]:
                cw = c1 - c0
                zp = z_part[(ci, c0)]
                for eb in range(DT):
                    pool = ps_mm if eb % 2 == 0 else ps_o
                    tag = "ps" if eb % 2 == 0 else "o"
                    ps = pool.tile([128, CH], F32, tag=tag, name=f"ps_z{ci}_{eb}")
                    for ct in range(EL // 128):
                        nc.tensor.matmul(
                            ps[:, 0:cw],
                            wp_sb[:, ct * D + eb * 128:ct * D + (eb + 1) * 128],
                            y_sb[:, ct * CH + c0:ct * CH + c1],
                            start=(ct == 0), stop=(ct == EL // 128 - 1))
                    zev = ppool.tile([128, CH], BF16, tag="zev", name=f"z{ci}_{eb}")
                    if eb % 2 == 0:
                        nc.gpsimd.tensor_copy(zev[:, 0:cw], ps[:, 0:cw])
                    else:
                        nc.vector.tensor_copy(zev[:, 0:cw], ps[:, 0:cw])
                    nc.sync.dma_start(zp[eb * 128:(eb + 1) * 128, :], zev[:, 0:cw])
                zr = z_rs[(ci, c0)]
                nc.gpsimd.collective_compute(
                    "ReduceScatter", mybir.AluOpType.add,
                    replica_groups=[[0, 1, 2, 3], [4, 5, 6, 7]],
                    ins=[zp.opt()], outs=[zr[:, 0:cw]])

        # ---------------- main loop ----------------
        wp_sb = None
        for ci in range(NCH):
            q_sb = qkv_chunk(ci, xh_cur, xl_cur)
            if ci + 1 < NCH:
                xh_cur, xl_cur = load_x(ci + 1)
            if ci == 0:
                wp_sb = load_wp("wp", wp, EL // 128, D, 2)
            y_sb = attn_chunk(ci, q_sb)
            proj_chunk(ci, y_sb, wp_sb)
        # RS cannot target an ExternalOutput; DRAM->DRAM DMAs move the
        # scattered slices into the output tensor. Emitted at the very end so
        # their RS-completion waits never head-of-line-block the SP DMA queue:
        # bounces 0..2 fire immediately, only the last waits on RS(3).
        for ci in range(NCH):
            for (c0, c1) in z_shapes[ci]:
                nc.sync.dma_start(zTc[ci * EL:(ci + 1) * EL, c0:c1],
                                  z_rs[(ci, c0)][:, 0:c1 - c0])
    nc.compile()
    return nc


def _tables():
    inv_freq = 1.0 / (ROPE_THETA ** (np.arange(0, HD, 2, dtype=np.float64) / HD))
    pos = np.arange(S, dtype=np.float64)
    f_half = np.outer(inv_freq, pos)                  # [64, S]
    freqs = np.concatenate([f_half, f_half], axis=0)  # [HD, S]
    emb32 = freqs.astype(np.float32)
    cos_t = np.cos(emb32) / QKV_SCALE
    sin_t = np.sin(emb32) / QKV_SCALE
    sgn = np.where(np.arange(HD) < HD // 2, -1.0, 1.0).astype(np.float32)[:, None]
    return cos_t.astype(ml_dtypes.bfloat16), (sin_t * sgn).astype(ml_dtypes.bfloat16)


def _split8(t, scale):
    """Split scale*t into fp8 hi + lo with hi = fp8(scale*t)."""
    t = np.asarray(t, np.float32) * np.float32(scale)
    hi = t.astype(ml_dtypes.float8_e4m3)
    lo = (t - hi.astype(np.float32)).astype(ml_dtypes.float8_e4m3)
    return np.ascontiguousarray(hi), np.ascontiguousarray(lo)


_NC_CACHE = {}


def _get_nc():
    if "nc" not in _NC_CACHE:
        _NC_CACHE["nc"] = _build()
    return _NC_CACHE["nc"]


def make_in_maps(x, W_attn, W_proj):
    x = np.asarray(x, dtype=np.float32)
    W_attn = np.asarray(W_attn, dtype=np.float32)
    W_proj = np.asarray(W_proj, dtype=np.float32)
    cos_t, sin_t = _tables()
    tri = np.triu(np.ones((128, 128), np.float32)).astype(ml_dtypes.bfloat16)
    ones = np.ones((128, 1), ml_dtypes.bfloat16)
    xb = [_split8(x[b].T, SX) for b in range(B)]
    wspl = {}
    for g in range(HPC):
        wspl[g] = {
            "wq": _split8(W_attn[:, g * EL:(g + 1) * EL], SW),
            "wk": _split8(W_attn[:, D + g * EL:D + (g + 1) * EL], SW),
            "wv": _split8(W_attn[:, 2 * D + g * EL:2 * D + (g + 1) * EL], SW),
        }
    in_maps = []
    for c in range(N_CORES):
        b, g = divmod(c, HPC)
        in_maps.append({
            "xh": xb[b][0], "xl": xb[b][1],
            "wqh": wspl[g]["wq"][0], "wql": wspl[g]["wq"][1],
            "wkh": wspl[g]["wk"][0], "wkl": wspl[g]["wk"][1],
            "wvh": wspl[g]["wv"][0], "wvl": wspl[g]["wv"][1],
            "wp": np.ascontiguousarray(
                W_proj[g * EL:(g + 1) * EL, :]).astype(ml_dtypes.bfloat16),
            "cosT": cos_t, "sinT": sin_t,
            "tri": tri, "ones": ones,
        })
    return in_maps


def assemble(results):
    out = np.empty((B, S, D), dtype=np.float32)
    for c in range(N_CORES):
        b, g = divmod(c, HPC)
        z = np.asarray(results[c]["zTc"]).astype(np.float32)   # [NCH*EL, CH]
        for ci in range(NCH):
            out[b, ci * CH:(ci + 1) * CH, g * EL:(g + 1) * EL] = \
                z[ci * EL:(ci + 1) * EL, :].T
    return out


def kernel(x, W_attn, W_proj):
    nc = _get_nc()
    in_maps = make_in_maps(x, W_attn, W_proj)
    res = bass_utils.run_bass_kernel_spmd(
        nc, in_maps, core_ids=list(range(N_CORES)), trace=False)
    return assemble(res.results)


if __name__ == "__main__":
    rng = np.random.default_rng(0)
    x = rng.standard_normal((B, S, D)).astype(np.float32)
    W_attn = (rng.standard_normal((D, 3 * D)) * D ** -0.5).astype(np.float32)
    W_proj = (rng.standard_normal((D, D)) * D ** -0.5).astype(np.float32)
    out = kernel(x, W_attn, W_proj)
    print("out", out.shape, out.dtype, np.abs(out).mean())



# revision 27
# speedup vs baseline: 1.1579x; 1.0077x over previous
"""Causal self-attention with RoPE on 8 TRN2 NeuronCores — v3.

Sharding: core c -> (batch b = c//4, head-group g = c%4; 4 heads of 128 each).
Tensor-parallel over heads x data-parallel over batch.

v3 strategy vs v2:
  - QKV projections in fp8 (e4m3) DoubleRow matmuls with hi/lo error
    compensation: x and W are split on the host into fp8 hi + lo parts;
    W*x = Wh*xh + Wh*xl + Wl*xh, each DoubleRow packing a k-tile pair.
    Same accuracy as bf16 at a fraction of the PE time.
  - rowsum reoriented: p is the stationary matmul operand against a ones
    column, giving [128q, 1] outputs (tiny moving dim) accumulated in one
    psum bank; rinv is rebuilt into a row via a flattening DMA.
  - next chunk's QKV emitted interleaved between attention tiles so the
    (in-order) PE queue has filler work while exp chases on ACT.
  - psum evacuations spread over gpsimd/DVE; ACT runs exp only.
  - last chunk's projection split by heads: heads 0-2 partial-projected
    during head 3's attention, shortening the end-of-kernel tail.
  - ReduceScatter outputs land in padded DRAM scratch.
"""
from contextlib import ExitStack

import numpy as np
import ml_dtypes

import concourse.bass as bass
import concourse.tile as tile
import concourse.mybir as mybir
from concourse import bacc, bass_utils

B = 2
S = 2048
D = 2048
NH, HD = 16, 128
HPC = 4                 # heads per core
EL = HPC * HD           # 512: local e-width per core
CH = 512                # token-chunk width
NCH = S // CH           # 4
DT = D // 128           # 16 d-tiles
ROPE_THETA = 10000.0
N_CORES = 8

F32 = mybir.dt.float32
F32R = mybir.dt.float32r
BF16 = mybir.dt.bfloat16
FP8 = mybir.dt.float8e4
AF = mybir.ActivationFunctionType
DR = mybir.MatmulPerfMode.DoubleRow

SX = 4.0      # fp8 scale for x
SW = 32.0     # fp8 scale for W_attn slices
QKV_SCALE = SX * SW          # q/k/v psums come out at 128x true scale
EXP_SCALE = float(HD ** -0.5)


def _build():
    nc = bacc.Bacc("TRN2", target_bir_lowering=False, debug=False,
                   enable_asserts=True, num_devices=N_CORES)
    xh = nc.dram_tensor("xh", [D, S], FP8, kind="ExternalInput").ap()
    xl = nc.dram_tensor("xl", [D, S], FP8, kind="ExternalInput").ap()
    wqh = nc.dram_tensor("wqh", [D, EL], FP8, kind="ExternalInput").ap()
    wql = nc.dram_tensor("wql", [D, EL], FP8, kind="ExternalInput").ap()
    wkh = nc.dram_tensor("wkh", [D, EL], FP8, kind="ExternalInput").ap()
    wkl = nc.dram_tensor("wkl", [D, EL], FP8, kind="ExternalInput").ap()
    wvh = nc.dram_tensor("wvh", [D, EL], FP8, kind="ExternalInput").ap()
    wvl = nc.dram_tensor("wvl", [D, EL], FP8, kind="ExternalInput").ap()
    wp = nc.dram_tensor("wp", [EL, D], BF16, kind="ExternalInput").ap()
    cosT = nc.dram_tensor("cosT", [HD, S], BF16, kind="ExternalInput").ap()
    sinT = nc.dram_tensor("sinT", [HD, S], BF16, kind="ExternalInput").ap()
    tri = nc.dram_tensor("tri", [128, 128], BF16, kind="ExternalInput").ap()
    ones = nc.dram_tensor("ones", [128, 1], BF16, kind="ExternalInput").ap()
    zTc = nc.dram_tensor("zTc", [NCH * EL, CH], BF16, kind="ExternalOutput").ap()

    with tile.TileContext(nc) as tc, \
         nc.allow_low_precision(reason="bf16 attention"), ExitStack() as ctx:
        # ---------------- pools ----------------
        cpool = ctx.enter_context(tc.tile_pool(name="const", bufs=1))
        wpool = ctx.enter_context(tc.tile_pool(name="w", bufs=1))
        xpool = ctx.enter_context(tc.tile_pool(name="x", bufs=2))
        kvres = ctx.enter_context(tc.tile_pool(name="kv", bufs=1))
        qpool = ctx.enter_context(tc.tile_pool(name="q", bufs=2))
        rope = ctx.enter_context(tc.tile_pool(name="rope", bufs=4))
        ppool = ctx.enter_context(tc.tile_pool(name="p", bufs=6))
        ypool = ctx.enter_context(tc.tile_pool(name="y", bufs=2))
        zpool = ctx.enter_context(tc.tile_pool(name="zacc", bufs=1))
        rpool = ctx.enter_context(tc.tile_pool(name="r", bufs=2))
        bpool = ctx.enter_context(tc.tile_pool(name="rbc", bufs=1))
        dram = ctx.enter_context(tc.tile_pool(name="dram", bufs=1, space="DRAM"))
        ps_mm = ctx.enter_context(tc.tile_pool(name="ps_mm", bufs=1, space="PSUM"))
        ps_q = ctx.enter_context(tc.tile_pool(name="ps_q", bufs=1, space="PSUM"))
        ps_s = ctx.enter_context(tc.tile_pool(name="ps_s", bufs=3, space="PSUM"))
        ps_o = ctx.enter_context(tc.tile_pool(name="ps_o", bufs=2, space="PSUM"))
        ps_r = ctx.enter_context(tc.tile_pool(name="ps_r", bufs=1, space="PSUM"))

        # ------------- weight / x loaders (split DMAs for pipelining) -------
        WSPLIT = 4            # d-tiles per weight sub-DMA

        def load_w(name, src, dt_, nsub):
            t = wpool.tile([128, DT * EL], FP8, name=name)
            step = DT // nsub
            for i in range(nsub):
                nc.sync.dma_start(
                    t[:, i * step * EL:(i + 1) * step * EL]
                        .rearrange("p (t e) -> p t e", t=step),
                    src.rearrange("(t p) e -> p t e", p=128)[:, i * step:(i + 1) * step, :])
            return t

        def load_wp(name, src, nt, wcols, nsub):
            t = wpool.tile([128, nt * wcols], BF16, name=name)
            step = nt // nsub
            for i in range(nsub):
                nc.sync.dma_start(
                    t[:, i * step * wcols:(i + 1) * step * wcols]
                        .rearrange("p (t e) -> p t e", t=step),
                    src.rearrange("(t p) e -> p t e", p=128)[:, i * step:(i + 1) * step, :])
            return t

        def load_x(ci):
            xht = xpool.tile([128, DT * CH], FP8, tag="xh", name=f"xh{ci}")
            xlt = xpool.tile([128, DT * CH], FP8, tag="xl", name=f"xl{ci}")
            nsub, step = 2, DT // 2
            for t, src in ((xht, xh), (xlt, xl)):
                for i in range(nsub):
                    nc.sync.dma_start(
                        t[:, i * step * CH:(i + 1) * step * CH]
                            .rearrange("p (t c) -> p t c", t=step),
                        src.rearrange("(t p) s -> p t s", p=128)
                          [:, i * step:(i + 1) * step, ci * CH:(ci + 1) * CH])
            return xht, xlt

        # startup order: wkh/xh first (the hi*hi matmuls need only those),
        # then wkl/xl, cos/sin (K rope), wq, wv, attention constants.
        wkh_sb = wpool.tile([128, DT * EL], FP8, name="wkh_sb")
        wkl_sb = wpool.tile([128, DT * EL], FP8, name="wkl_sb")
        xh_cur = xpool.tile([128, DT * CH], FP8, tag="xh", name="xh0")
        xl_cur = xpool.tile([128, DT * CH], FP8, tag="xl", name="xl0")
        for (i0, i1) in [(0, 2), (2, 5), (5, 8), (8, 12), (12, 16)]:
            nc.sync.dma_start(
                wkh_sb[:, i0 * EL:i1 * EL].rearrange("p (t e) -> p t e", t=i1 - i0),
                wkh.rearrange("(t p) e -> p t e", p=128)[:, i0:i1, :])
            nc.sync.dma_start(
                xh_cur[:, i0 * CH:i1 * CH].rearrange("p (t c) -> p t c", t=i1 - i0),
                xh.rearrange("(t p) s -> p t s", p=128)[:, i0:i1, 0:CH])
            nc.sync.dma_start(
                wkl_sb[:, i0 * EL:i1 * EL].rearrange("p (t e) -> p t e", t=i1 - i0),
                wkl.rearrange("(t p) e -> p t e", p=128)[:, i0:i1, :])
            nc.sync.dma_start(
                xl_cur[:, i0 * CH:i1 * CH].rearrange("p (t c) -> p t c", t=i1 - i0),
                xl.rearrange("(t p) s -> p t s", p=128)[:, i0:i1, 0:CH])
        cos_t = cpool.tile([HD, S], BF16)
        nc.sync.dma_start(cos_t[:], cosT)
        sin_t = cpool.tile([HD, S], BF16)
        nc.sync.dma_start(sin_t[:], sinT)
        wqh_sb = load_w("wqh", wqh, FP8, WSPLIT)
        wql_sb = load_w("wql", wql, FP8, WSPLIT)
        wvh_sb = load_w("wvh", wvh, FP8, WSPLIT)
        wvl_sb = load_w("wvl", wvl, FP8, WSPLIT)
        tri_t = cpool.tile([128, 128], BF16)
        nc.sync.dma_start(tri_t[:], tri)
        ones_t = cpool.tile([128, 1], BF16)
        nc.sync.dma_start(ones_t[:], ones)

        # ---------------- persistent K / V, z scratch ----------------
        k_c = [kvres.tile([HD, HPC * CH], BF16, name=f"k{ci}") for ci in range(NCH)]
        v_t = [kvres.tile([128, EL], BF16, name=f"v{st}") for st in range(S // 128)]
        z_part = {}
        z_rs = {}
        for ci in range(NCH):
            z_part[ci] = dram.tile([D, CH], BF16, tag=f"zp{ci}", name=f"zp{ci}")
            # padded so the RS writes a strided row-major region
            z_rs[ci] = dram.tile([EL, CH + 64], BF16, tag=f"zr{ci}", name=f"zr{ci}")

        _QKV_POOLS = [(ps_s, "s_ps"), (ps_o, "o"), (ps_mm, "ps"), (ps_q, "q_ps")]
        _qkv_rot = [0]

        def qkv_psum_bulk(name):
            pool, tag = _QKV_POOLS[_qkv_rot[0] % 4]
            _qkv_rot[0] += 1
            return pool.tile([128, CH], F32, tag=tag, name=name)

        _il_rot = [0]

        def qkv_psum_il(name):
            pool, tag = (ps_q, "q_ps") if _il_rot[0] % 2 == 0 else (ps_mm, "ps")
            _il_rot[0] += 1
            return pool.tile([128, CH], F32, tag=tag, name=name)

        def rope_tail(ci, h, ps, out_ap, tagp):
            """psum [HD, CH] (at QKV_SCALE) -> RoPE -> out_ap (true scale via
            the 1/QKV_SCALE folded into the cos/sin tables)."""
            pre = rope.tile([HD, CH], BF16, tag="pre", name=f"pre_{tagp}")
            nc.gpsimd.tensor_copy(pre[:], ps[:])
            rot = rope.tile([HD, CH], BF16, tag="rot", name=f"rot_{tagp}")
            nc.sync.dma_start(rot[0:64, :], pre[64:128, :])
            nc.sync.dma_start(rot[64:128, :], pre[0:64, :])
            cs = cos_t[:, ci * CH:(ci + 1) * CH]
            sn = sin_t[:, ci * CH:(ci + 1) * CH]
            t1 = rope.tile([HD, CH], BF16, tag="t1", name=f"t1_{tagp}")
            t2 = rope.tile([HD, CH], BF16, tag="t2", name=f"t2_{tagp}")
            nc.vector.tensor_mul(t1[:], pre[:], cs)
            nc.vector.tensor_mul(t2[:], rot[:], sn)
            nc.vector.tensor_add(out_ap, t1[:], t2[:])

        def gen_kq_unit(ci, h, x_hi, x_lo, w_hi, w_lo, out_ap, tagp, psup):
            """Generator: one head's fp8 hi/lo DR projection + rope.
            Yields every few matmuls so it can interleave with attention."""
            ps = psup(f"ps_{tagp}")
            wh = w_hi.rearrange("p (t e) -> p t e", t=DT)
            wl = w_lo.rearrange("p (t e) -> p t e", t=DT)
            xhr = x_hi.rearrange("p (t c) -> p t c", t=DT)
            xlr = x_lo.rearrange("p (t c) -> p t c", t=DT)
            hs = slice(h * HD, (h + 1) * HD)
            NP = DT // 2
            n = 0
            for term, (wt, xt) in enumerate(((wh, xhr), (wh, xlr), (wl, xhr))):
                for jj in range(NP):
                    ts = slice(2 * jj, 2 * jj + 2)
                    nc.tensor.matmul(
                        ps[:], wt[:, ts, hs], xt[:, ts, :],
                        start=(term == 0 and jj == 0),
                        stop=(term == 2 and jj == NP - 1), perf_mode=DR)
                    n += 1
                    if n % 4 == 0:
                        yield
            rope_tail(ci, h, ps, out_ap, tagp)
            yield

        def gen_v_unit(ci, st, x_hi, x_lo, psup):
            ps = psup(f"ps_v{ci}_{st}")
            wvh_r = wvh_sb.rearrange("p (t e) -> p t e", t=DT)
            wvl_r = wvl_sb.rearrange("p (t e) -> p t e", t=DT)
            xh_r = x_hi.rearrange("p (t c) -> p t c", t=DT)
            xl_r = x_lo.rearrange("p (t c) -> p t c", t=DT)
            ss = slice(st * 128, (st + 1) * 128)
            NP = DT // 2
            n = 0
            for term, (xt, wt) in enumerate(
                    ((xh_r, wvh_r), (xl_r, wvh_r), (xh_r, wvl_r))):
                for jj in range(NP):
                    ts = slice(2 * jj, 2 * jj + 2)
                    nc.tensor.matmul(
                        ps[:], xt[:, ts, ss], wt[:, ts, :],
                        start=(term == 0 and jj == 0),
                        stop=(term == 2 and jj == NP - 1), perf_mode=DR)
                    n += 1
                    if n % 4 == 0:
                        yield
            # v psum is at QKV_SCALE; rescale to true scale on evac
            nc.gpsimd.tensor_scalar_mul(v_t[ci * 4 + st][:], ps[:],
                                        1.0 / QKV_SCALE)
            yield

        def qkv_units(ci, x_hi, x_lo, psup):
            q_sb = qpool.tile([128, HPC * CH], BF16, tag="q", name=f"q{ci}")
            gens = []
            for h in range(HPC):
                gens.append(gen_kq_unit(ci, h, x_hi, x_lo, wkh_sb, wkl_sb,
                                        k_c[ci][:, h * CH:(h + 1) * CH],
                                        f"k{ci}_{h}", psup))
            for h in range(HPC):
                gens.append(gen_kq_unit(ci, h, x_hi, x_lo, wqh_sb, wql_sb,
                                        q_sb[:, h * CH:(h + 1) * CH],
                                        f"q{ci}_{h}", psup))
            for st in range(CH // 128):
                gens.append(gen_v_unit(ci, st, x_hi, x_lo, psup))
            return q_sb, gens

        class GQ:
            """A queue of emit-generators, pumped between attention tiles."""

            def __init__(self, gens=()):
                self.gens = list(gens)

            def pump(self, n):
                while n > 0 and self.gens:
                    try:
                        next(self.gens[0])
                        n -= 1
                    except StopIteration:
                        self.gens.pop(0)

            def drain(self):
                while self.gens:
                    try:
                        next(self.gens[0])
                    except StopIteration:
                        self.gens.pop(0)

        # ---------------- projection ----------------
        _PROJ_POOLS = [(ps_mm, "ps"), (ps_o, "o"), (ps_q, "q_ps"), (ps_o, "o")]

        def gen_proj(ci, y_sb, wp_sb, cts, zacc, mode, pools):
            """Partial projection over head-tiles `cts` of chunk ci.
            mode: 'full' (evac+dma), 'acc0' (write zacc), 'accN' (add zacc),
            'fin' (add zacc, evac+dma)."""
            zp = z_part[ci]
            for eb in range(DT):
                pool, tag = pools[eb % len(pools)]
                ps = pool.tile([128, CH], F32, tag=tag, name=f"ps_z{ci}_{eb}_{mode}")
                for i, ct in enumerate(cts):
                    nc.tensor.matmul(
                        ps[:], wp_sb[:, ct * D + eb * 128:ct * D + (eb + 1) * 128],
                        y_sb[:, ct * CH:(ct + 1) * CH],
                        start=(i == 0), stop=(i == len(cts) - 1))
                if mode == "acc0":
                    if eb % 2 == 0:
                        nc.gpsimd.tensor_copy(zacc[:, eb * CH:(eb + 1) * CH], ps[:])
                    else:
                        nc.vector.tensor_copy(zacc[:, eb * CH:(eb + 1) * CH], ps[:])
                elif mode == "accN":
                    za = zacc[:, eb * CH:(eb + 1) * CH]
                    if eb % 2 == 0:
                        nc.gpsimd.tensor_add(za, za, ps[:])
                    else:
                        nc.vector.tensor_add(za, za, ps[:])
                else:
                    zev = ppool.tile([128, CH], BF16, tag="zev",
                                     name=f"z{ci}_{eb}_{mode}")
                    if mode == "fin":
                        if eb % 2 == 0:
                            nc.gpsimd.tensor_add(zev[:], zacc[:, eb * CH:(eb + 1) * CH], ps[:])
                        else:
                            nc.vector.tensor_add(zev[:], zacc[:, eb * CH:(eb + 1) * CH], ps[:])
                    else:
                        if eb % 2 == 0:
                            nc.gpsimd.tensor_copy(zev[:], ps[:])
                        else:
                            nc.vector.tensor_copy(zev[:], ps[:])
                    nc.sync.dma_start(zp[eb * 128:(eb + 1) * 128, :], zev[:])
                yield

        def emit_rs(ci):
            nc.gpsimd.collective_compute(
                "ReduceScatter", mybir.AluOpType.add,
                replica_groups=[[0, 1, 2, 3], [4, 5, 6, 7]],
                ins=[z_part[ci].opt()], outs=[z_rs[ci][:, 0:CH]])

        # ---------------- attention ----------------
        def attn_chunk(ci, q_sb, filler, y_sb, on_head_done):
            """Causal attention for query chunk ci over key chunks 0..ci.
            `filler` (a GQ) is pumped between tiles to keep PE busy while
            exp chases on ACT."""
            n_jt = 4 * ci + 4
            tiles = [(h, jt) for h in range(HPC) for jt in range(n_jt)]
            state = {}
            pending = []

            def emit_or(ent):
                h, jt, p, off = ent
                o_ps, r_ps = state[h]
                nc.tensor.matmul(
                    o_ps[:, off:], v_t[jt][:, h * HD:(h + 1) * HD],
                    p[:, off:], start=(jt == 0), stop=(jt == n_jt - 1))
                # rowsum with p stationary: out [128q, 1] per query slice
                for qs in range(off // 128, 4):
                    nc.tensor.matmul(
                        r_ps[:, qs:qs + 1], p[:, qs * 128:(qs + 1) * 128],
                        ones_t[:], start=(jt == 0 and qs == 0),
                        stop=(jt == n_jt - 1 and qs == 3))
                if jt == n_jt - 1:
                    # normalize head h: y = o * (1/rowsum); rinv arrives with
                    # queries on partitions, flattened to a row by DMA, then
                    # broadcast; the (q,j) interleave is undone by the read AP
                    rinv = rpool.tile([128, 4], BF16, tag="rinv")
                    nc.vector.reciprocal(rinv[:], r_ps[:, 0:4])
                    rrow = rpool.tile([1, CH], BF16, tag="rrow",
                                      name=f"rr{ci}_{h}")
                    nc.sync.dma_start(rrow[:], rinv[:])
                    rbc = bpool.tile([128, CH], BF16, tag="rbc", name=f"rb{ci}_{h}")
                    nc.gpsimd.partition_broadcast(rbc[:], rrow[:])
                    nc.vector.tensor_mul(
                        y_sb[:, h * CH:(h + 1) * CH], o_ps[:],
                        rbc.rearrange("p (q j) -> p j q", j=4))
                    del state[h]
                    on_head_done(h)

            n_t = len(tiles)
            for idx, (h, jt) in enumerate(tiles):
                if jt == 0:
                    o_ps = ps_o.tile([HD, CH], F32, tag="o", name=f"o{ci}_{h}")
                    r_ps = ps_r.tile([128, CH], F32, tag="r", name=f"r{ci}_{h}")
                    state[h] = (o_ps, r_ps)
                diag = jt - 4 * ci
                off = 128 * diag if diag > 0 else 0
                cj, j2 = divmod(jt, 4)
                s_ps = ps_s.tile([128, CH], F32, tag="s_ps", name=f"s{ci}_{h}_{jt}")
                nc.tensor.matmul(
                    s_ps[:, off:], k_c[cj][:, h * CH + j2 * 128:h * CH + (j2 + 1) * 128],
                    q_sb[:, h * CH + off:(h + 1) * CH], start=True, stop=True)
                p = ppool.tile([128, CH], BF16, tag="p")
                nc.scalar.activation(p[:, off:], s_ps[:, off:], AF.Exp,
                                     scale=EXP_SCALE)
                if 0 <= diag:
                    nc.vector.tensor_mul(
                        p[:, off:off + 128], p[:, off:off + 128], tri_t[:])
                if len(pending) >= 3:
                    emit_or(pending.pop(0))
                pending.append((h, jt, p, off))
                filler.pump(max(1, (len(filler.gens) * 7) // max(1, n_t - idx)))
            for ent in pending:
                emit_or(ent)

        # ---------------- main loop ----------------
        q_cur, gens0 = qkv_units(0, xh_cur, xl_cur, qkv_psum_bulk)
        GQ(gens0).drain()
        xh_cur, xl_cur = load_x(1)
        wp_sb = load_wp("wp", wp, EL // 128, D, 2)

        for ci in range(NCH):
            y_sb = ypool.tile([128, HPC * CH], BF16, tag="y", name=f"y{ci}")
            last = ci == NCH - 1
            filler = GQ()
            q_next = None
            if not last:
                q_next, gens = qkv_units(ci + 1, xh_cur, xl_cur, qkv_psum_il)
                filler.gens.extend(gens)
            zacc = None
            if last:
                zacc = zpool.tile([128, DT * CH], BF16, name="zacc")

            def on_head_done(h, ci=ci, y_sb=y_sb, zacc=zacc, filler=filler,
                             last=last):
                if last and h == 2:
                    filler.gens.append(
                        gen_proj(ci, y_sb, wp_sb, (0, 1, 2), zacc, "acc0",
                                 [(ps_mm, "ps"), (ps_q, "q_ps")]))

            attn_chunk(ci, q_cur, filler, y_sb, on_head_done)
            filler.drain()
            if last:
                GQ([gen_proj(ci, y_sb, wp_sb, (3,), zacc, "fin",
                             _PROJ_POOLS)]).drain()
            else:
                GQ([gen_proj(ci, y_sb, wp_sb, (0, 1, 2, 3), None, "full",
                             _PROJ_POOLS)]).drain()
            emit_rs(ci)
            if ci + 2 <= NCH - 1:
                xh_cur, xl_cur = load_x(ci + 2)
            q_cur = q_next

        # RS cannot target an ExternalOutput; DRAM->DRAM DMAs move the
        # scattered slices into the output tensor. Emitted at the very end so
        # their RS-completion waits never head-of-line-block the SP DMA queue.
        for ci in range(NCH):
            nc.sync.dma_start(zTc[ci * EL:(ci + 1) * EL, :],
                              z_rs[ci][:, 0:CH])
    nc.compile()
    return nc


def _tables():
    inv_freq = 1.0 / (ROPE_THETA ** (np.arange(0, HD, 2, dtype=np.float64) / HD))
    pos = np.arange(S, dtype=np.float64)
    f_half = np.outer(inv_freq, pos)                  # [64, S]
    freqs = np.concatenate([f_half, f_half], axis=0)  # [HD, S]
    emb32 = freqs.astype(np.float32)
    cos_t = np.cos(emb32) / QKV_SCALE
    sin_t = np.sin(emb32) / QKV_SCALE
    sgn = np.where(np.arange(HD) < HD // 2, -1.0, 1.0).astype(np.float32)[:, None]
    return cos_t.astype(ml_dtypes.bfloat16), (sin_t * sgn).astype(ml_dtypes.bfloat16)


def _split8(t, scale):
    """Split scale*t into fp8 hi + lo with hi = fp8(scale*t)."""
    t = np.asarray(t, np.float32) * np.float32(scale)
    hi = t.astype(ml_dtypes.float8_e4m3)
    lo = (t - hi.astype(np.float32)).astype(ml_dtypes.float8_e4m3)
    return np.ascontiguousarray(hi), np.ascontiguousarray(lo)


_NC_CACHE = {}


def _get_nc():
    if "nc" not in _NC_CACHE:
        _NC_CACHE["nc"] = _build()
    return _NC_CACHE["nc"]


def make_in_maps(x, W_attn, W_proj):
    x = np.asarray(x, dtype=np.float32)
    W_attn = np.asarray(W_attn, dtype=np.float32)
    W_proj = np.asarray(W_proj, dtype=np.float32)
    cos_t, sin_t = _tables()
    tri = np.triu(np.ones((128, 128), np.float32)).astype(ml_dtypes.bfloat16)
    ones = np.ones((128, 1), ml_dtypes.bfloat16)
    xb = [_split8(x[b].T, SX) for b in range(B)]
    wspl = {}
    for g in range(HPC):
        wspl[g] = {
            "wq": _split8(W_attn[:, g * EL:(g + 1) * EL], SW),
            "wk": _split8(W_attn[:, D + g * EL:D + (g + 1) * EL], SW),
            "wv": _split8(W_attn[:, 2 * D + g * EL:2 * D + (g + 1) * EL], SW),
        }
    in_maps = []
    for c in range(N_CORES):
        b, g = divmod(c, HPC)
        in_maps.append({
            "xh": xb[b][0], "xl": xb[b][1],
            "wqh": wspl[g]["wq"][0], "wql": wspl[g]["wq"][1],
            "wkh": wspl[g]["wk"][0], "wkl": wspl[g]["wk"][1],
            "wvh": wspl[g]["wv"][0], "wvl": wspl[g]["wv"][1],
            "wp": np.ascontiguousarray(
                W_proj[g * EL:(g + 1) * EL, :]).astype(ml_dtypes.bfloat16),
            "cosT": cos_t, "sinT": sin_t,
            "tri": tri, "ones": ones,
        })
    return in_maps


def assemble(results):
    out = np.empty((B, S, D), dtype=np.float32)
    for c in range(N_CORES):
        b, g = divmod(c, HPC)
        z = np.asarray(results[c]["zTc"]).astype(np.float32)   # [NCH*EL, CH]
        for ci in range(NCH):
            out[b, ci * CH:(ci + 1) * CH, g * EL:(g + 1) * EL] = \
                z[ci * EL:(ci + 1) * EL, :].T
    return out


def kernel(x, W_attn, W_proj):
    nc = _get_nc()
    in_maps = make_in_maps(x, W_attn, W_proj)
    res = bass_utils.run_bass_kernel_spmd(
        nc, in_maps, core_ids=list(range(N_CORES)), trace=False)
    return assemble(res.results)


if __name__ == "__main__":
    rng = np.random.default_rng(0)
    x = rng.standard_normal((B, S, D)).astype(np.float32)
    W_attn = (rng.standard_normal((D, 3 * D)) * D ** -0.5).astype(np.float32)
    W_proj = (rng.standard_normal((D, D)) * D ** -0.5).astype(np.float32)
    out = kernel(x, W_attn, W_proj)
    print("out", out.shape, out.dtype, np.abs(out).mean())
